# revision 47
# baseline (speedup 1.0000x reference)
"""MultiHeadAttention (B=2, S=2048, d_model=1024, 16 heads, causal) on 8 TRN2 cores.

Sharding: core i handles batch (i//4) and heads 4*(i%4) .. 4*(i%4)+4 (tensor
parallel over heads within a batch).  Each core computes its 4 heads'
Q/K/V projections, causal attention, and the partial output projection
(contribution of its 256 head-dims to all 1024 output dims).  The host sums
the 4 bf16 partials per batch in float64 and adds the output bias.

Precision strategy (validated against the fp32 reference in numpy; the
harness gate is scale-relative absmax 2e-2, this kernel lands ~9.1e-3):
  - everything touching QUERIES < 512 (q-tile 0) runs bf16: those rows
    average over too few keys to absorb fp8 quantization noise (q=0 outputs
    V[0] exactly, so fp8's ~6% element error would land raw in the output).
  - tiles 1-3 run fp8e4m3 end to end: x tiles 1-3 (plus an fp8 copy of
    x tile 0 that feeds the fp8 K chunks 0-3 those tiles read, so their
    early pairs never wait on the 1MB bf16 x0 transfer) and the
    Wq/Wk/Wv copies ship fp8 from the host; Q/K/V projections, QK^T scores, and the PV
    matmul all use fp8 DoubleRow perf mode, which the PE charges at 0.5
    cycles/row while contracting TWO 128-row k-tiles per instruction:
      * projections: c-chunk pairs packed -> 4x cheaper than bf16
      * PV: two key-chunks packed ([128, 2(parity), 4*68] fp8 V layout,
        head stride padded to 68 for the 16B outer-stride ISA rule) -> 4x
      * scores: d_k=64 only fills half the array, so the second k-tile is
        ZERO-padded (KT8/QT8 are [128, 2, 512] with dim1=1 memset once;
        0 * garbage would be NaN-poisoned otherwise) -> 2x
    Errors average out over >=512 keys: the fp8 kernel's absmax error is
    within 2.5x of the all-bf16 kernel's.
  - the output projection stays bf16 (fp8 fails the gate: contraction 256
    gives no averaging), PSUM accumulation is fp32 throughout, exp output
    is written directly as fp8 for tiles >= 1 (free on ACT).
  - bias handling: 1/sqrt(d_k) folds into Wq/bq on the host; V's bias rides
    a broadcast row of the bqkv tensor; the softmax denominator comes from
    a ones-column appended to V (accumulated by the same PV matmuls).

The machine balance after fp8: ACT (exp) ~77.5us busy is the bottleneck
(PE ~59, DVE ~60, Pool ~43), so the whole kernel is scheduled as ONE
global pipeline of 40 exp-paced stages (one per (tile, head-pair,
key-chunk-pair)):
  - tiles process in order [1, 2, 0, 3]: tile 1's fp8 weights + x land
    ~3us after launch (vs ~9us for the bf16 tile-0 supply), so the exp
    stream starts at ~8us instead of ~15us.  DMA order is hand-sequenced
    (fp8 Wqk whole -> x1 halves -> bqkv -> x0/WqkK interleaved -> ...);
    fp8 tensors are laid out so both DMA sides keep >=512B contiguous
    runs (256B runs pay a 2x descriptor-latency penalty).
  - PV flushes run a 7-deep software pipeline ACROSS group and tile
    boundaries: a group's flush + normalize tail overlaps the next
    group's scores/exp instead of serializing at each boundary.  A
    group's normalize MULTS are deferred one stage so the in-order DVE
    never head-of-line stalls on the Pool broadcast roundtrip.
  - projection/output-projection work drains into each stage under a PE
    budget (~1000ns) with per-item DMA-readiness stages (draining a
    not-yet-landed item would head-of-line stall the in-order PE), and
    keyed force points guarantee a group's Q/K/V exist exactly when its
    scores or flushes consume them.
  - causal masking: per-half column trims on the diagonal pairs plus
    gpsimd affine_selects on the two 128x128 triangle blocks (the fp8
    path widens one select to also zero the below-diagonal block that
    column ranges used to exclude).
  - the endgame pairs output rows into single DMAs (HWDGE costs 625ns
    per dma_start), alternates psum between the mm and then-idle pv
    pools, splits copies ACT/DVE, and quarter-splits the last normalize
    so the tail starts as early as possible.

Cost-model (TimelineSim) estimate: 100.96 us/core (fp32r baseline kernel:
146.0 us; bf16 predecessor: 130.6 us).  Scale-relative absmax error vs
the fp32 reference: 9.29e-3 (gate is 2e-2).
"""

import numpy as np
import ml_dtypes

import concourse.bass as bass
import concourse.tile as tile
import concourse.mybir as mybir
from concourse import bacc
from concourse.bass_utils import run_bass_kernel_spmd

dt = mybir.dt
AF = mybir.ActivationFunctionType
BF16 = ml_dtypes.bfloat16

D_MODEL = 1024
N_HEADS = 16
D_K = 64
B = 2
S = 2048
H_PER_CORE = 4
DH = H_PER_CORE * D_K  # 256
N_CORES = 8
CCH = D_MODEL // 128  # 8 contraction chunks
QT_TILES = S // 512  # 4
KCH = S // 128  # 16 key chunks
VW = D_K + 1  # 65
VWP = 68  # fp8 V per-head stride: DoubleRow needs 16B-aligned outer strides

_CACHE = {}

import os

CEXP = int(os.environ.get("K_CEXP", "0"))
SC_BUFS = int(os.environ.get("K_SC_BUFS", "2"))
MM_BUFS = 2
PV_BUFS = 1
PT_BUFS = int(os.environ.get("K_PT", "13"))
N_WARMUP = int(os.environ.get("K_WARMUP", "5"))
WARM_MEMSET = int(os.environ.get("K_WARM_MEMSET", "1"))
DRAIN2 = int(os.environ.get("K_DRAIN2", "0"))
FLUSH1 = int(os.environ.get("K_FLUSH1", "0"))
XPRE = int(os.environ.get("K_XPRE", "2"))
RESERVE = int(os.environ.get("K_RESERVE", "3"))
GDRAIN = int(os.environ.get("K_GDRAIN", "3"))
BUD_T0 = int(os.environ.get("K_BUD_T0", "700"))
BUD = int(os.environ.get("K_BUD", "1000"))
BUD_G = int(os.environ.get("K_BUD_G", "1300"))
OPROJ_COST = int(os.environ.get("K_OPROJ_COST", "480"))
N_FILLER = int(os.environ.get("K_FILLER", "0"))
NORM_ORDER = os.environ.get("K_NORM", "paired")
SPLIT_LAST = int(os.environ.get("K_SPLIT_LAST", "0"))
TAIL_PAIR = int(os.environ.get("K_TAIL_PAIR", "1"))
PACE_R = int(os.environ.get("K_PACE_R", "0"))
PV_DEPTH = int(os.environ.get("K_PV_DEPTH", "7"))
WV_SPLIT = int(os.environ.get("K_WV_SPLIT", "1"))
WV_EARLY = int(os.environ.get("K_WV_EARLY", "0"))
BQKV_EARLY = int(os.environ.get("K_BQKV_EARLY", "0"))
OSB_BUFS = int(os.environ.get("K_OSB", "6"))
SM_BUFS = int(os.environ.get("K_SM", "8"))
_oact = os.environ.get("K_OACT", "all")
OACT = {"all": lambda i: True, "mix": lambda i: i < 3 or i % 2 == 0,
        "alt": lambda i: i % 2 == 0}[_oact]
_usc = os.environ.get("K_USC", "none")
USC = {"none": lambda i: False, "alt": lambda i: i % 2 == 1,
       "all": lambda i: True}[_usc]


def _build():
    nc = bacc.Bacc("TRN2", target_bir_lowering=False, debug=False,
                   num_devices=N_CORES)

    # x tile 0 ships bf16 (tile-0 numerics need it); tiles 1-3 ship fp8.
    # Weights ship in both precisions: bf16 for the tile-0/chunk-0-3
    # projections, fp8 for the DoubleRow projections of everything else.
    xT0 = nc.dram_tensor("xT0", [D_MODEL, 512], dt.bfloat16,
                         kind="ExternalInput").ap()
    xT8 = nc.dram_tensor("xT8", [D_MODEL, S - 512], dt.float8e4,
                         kind="ExternalInput").ap()
    x08T = nc.dram_tensor("x08T", [D_MODEL, 512], dt.float8e4,
                          kind="ExternalInput").ap()
    WqkT = nc.dram_tensor("WqkT", [D_MODEL, 2 * DH], dt.bfloat16,
                          kind="ExternalInput").ap()
    Wqk8T = nc.dram_tensor("Wqk8T", [D_MODEL, 2 * DH], dt.float8e4,
                           kind="ExternalInput").ap()
    WvT = nc.dram_tensor("WvT", [D_MODEL, DH], dt.bfloat16, kind="ExternalInput").ap()
    Wv8T = nc.dram_tensor("Wv8T", [D_MODEL // 2, 2 * DH], dt.float8e4,
                          kind="ExternalInput").ap()
    WoT = nc.dram_tensor("WoT", [DH, D_MODEL], dt.bfloat16, kind="ExternalInput").ap()
    bqkv = nc.dram_tensor("bqkv", [128, 4 + DH], dt.float32,
                          kind="ExternalInput").ap()
    out = nc.dram_tensor("out", [S, D_MODEL], dt.bfloat16, kind="ExternalOutput").ap()


    with tile.TileContext(nc) as tc:
        with (
            tc.tile_pool(name="weights", bufs=1) as wpool,
            tc.tile_pool(name="acts", bufs=1) as apool,
            tc.tile_pool(name="pt", bufs=PT_BUFS) as ptpool,
            tc.tile_pool(name="pt8", bufs=PT_BUFS) as pt8pool,
            tc.tile_pool(name="sm", bufs=4) as smpool,
            tc.tile_pool(name="outsb", bufs=4) as opool,
            tc.tile_pool(name="mm", bufs=MM_BUFS, space="PSUM") as mmpool,
            tc.tile_pool(name="sc", bufs=SC_BUFS, space="PSUM") as scpool,
            tc.tile_pool(name="pv", bufs=1, space="PSUM") as pvpool,
        ):
            # ---- PE warmup: the cost model's p-state ramp needs ~3us of
            # continuous PE activity before matmuls run at full clock.  The
            # first input DMAs take ~1-2us to land, so burn that window with
            # dummy matmuls on a scratch tile (never read back); real matmuls
            # queue behind them and start fully warm.
            warm_sb = wpool.tile([128, 512], dt.bfloat16, name="warm",
                                 tag="warm")
            if WARM_MEMSET:
                nc.vector.memset(warm_sb[:], 0.0)
            warm_ps = mmpool.tile([128, 512], dt.float32, name="ps", tag="ps")
            for wi in range(N_WARMUP):
                nc.tensor.matmul(warm_ps[:], warm_sb[:, 0:128], warm_sb[:],
                                 start=True, stop=True, skip_group_check=True)

            # ---- input loads, ordered by first use under the [1,2,0,3]
            # tile processing order: the fp8 Wqk + fp8 x1 land first (~3us)
            # so tile 1's DoubleRow projections and first exp start early;
            # the bf16 Wqk/x0 (tile-0 path) stream in behind them.
            # tile0 x: [128, c(8), 512] bf16; tiles 1-3: [128, cp(4),
            # 2(c-parity), 512] fp8 -- the DoubleRow k-tile pair layout.
            xT_t = [wpool.tile([128, CCH, 512], dt.bfloat16, name="xTt0",
                               tag="xTt0") if t == 0 else
                    wpool.tile([128, CCH // 2, 2, 512], dt.float8e4,
                               name=f"xTt{t}", tag=f"xTt{t}")
                    for t in range(QT_TILES)]
            xT0_r = xT0.rearrange("(c p) q -> p c q", p=128)
            x8_r = xT8.rearrange("(cp i p) q -> p cp i q", p=128, i=2)
            x08_r = x08T.rearrange("(cp i p) q -> p cp i q", p=128, i=2)
            x08_sb = wpool.tile([128, CCH // 2, 2, 512], dt.float8e4,
                                name="x08", tag="x08")
            Wqk_sb = wpool.tile([128, CCH, 2 * DH], dt.bfloat16)
            Wqk8_sb = wpool.tile([128, CCH // 2, 2, 2 * DH], dt.float8e4)
            Wv_sb = wpool.tile([128, CCH, DH], dt.bfloat16)
            # fp8 Wv ships host-packed as [512, 2*DH] (two c-rows per DRAM
            # row) so both DMA sides have 512B contiguous runs -- 256B runs
            # pay a 2x DMA latency penalty
            Wv8_sb = wpool.tile([128, CCH // 2, 2 * DH], dt.float8e4)
            wqk_r = WqkT.rearrange("(c p) d -> p c d", p=128)
            wqk8_r = Wqk8T.rearrange("(cp i p) d -> p cp i d", p=128, i=2)
            wv_r = WvT.rearrange("(c p) d -> p c d", p=128)
            wv8_r = Wv8T.rearrange("(cp p) e -> p cp e", p=128)
            bqkv_sb = wpool.tile([128, 4 + DH], dt.float32)
            # head-critical: the j0 half of the fp8 Wqk (host layout is
            # [Qj0|Kj0|Qj1|Kj1] so it is contiguous), then fp8 x1 -- these
            # gate tile 1's first scores.  bqkv (bias, needed by the first
            # DVE write) follows, then the bf16 x0 (gates the chunk-0-3
            # K/V projections tile 1's later pairs read).
            nc.sync.dma_start(Wqk8_sb[:], wqk8_r)
            nc.sync.dma_start(xT_t[1][:, 0:2, :, :], x8_r[:, 0:2, :, 0:512])
            nc.sync.dma_start(xT_t[1][:, 2:4, :, :], x8_r[:, 2:4, :, 0:512])
            nc.sync.dma_start(bqkv_sb[:], bqkv)
            # fp8 copy of x tile 0: feeds the fp8 K chunks 0-3 that tiles
            # >=1 read, so their early pairs never wait on the 1MB bf16 x0
            nc.sync.dma_start(x08_sb[:], x08_r)
            nc.sync.dma_start(Wv8_sb[:], wv8_r)
            nc.sync.dma_start(Wv_sb[:, 0:4, :], wv_r[:, 0:4, :])
            nc.sync.dma_start(Wv_sb[:, 4:8, :], wv_r[:, 4:8, :])
            nc.sync.dma_start(xT_t[0][:, 0:4, :], xT0_r[:, 0:4, :])
            nc.sync.dma_start(xT_t[0][:, 4:8, :], xT0_r[:, 4:8, :])
            nc.sync.dma_start(Wqk_sb[:, :, 256:512], wqk_r[:, :, 256:512])
            bqs_sb = bqkv_sb[:, 0:2]
            bks_sb = bqkv_sb[:, 2:4]
            bvb_sb = bqkv_sb[:, 4:4 + DH]
            WoT_sb = [wpool.tile([128, D_MODEL], dt.bfloat16, name=f"Wo{j}",
                                 tag=f"Wo{j}") for j in range(2)]
            # remaining fp8 x tiles; needed from ~20us on
            for tx in (2, 3):
                nc.sync.dma_start(
                    xT_t[tx][:],
                    x8_r[:, :, :, (tx - 1) * 512:tx * 512])
            nc.sync.dma_start(Wqk_sb[:, :, 0:256], wqk_r[:, :, 0:256])

            # ---- fully streamed per q-tile: projections for tile t, then
            # attention for tile t (overlaps next tile's projections on PE),
            # then tile t's slice of the output projection.
            # bf16 Q/K only for tile 0 (its low-query rows need bf16
            # scores); fp8 zero-padded [128, 2(k-tile), 512] Q/K feed the
            # DoubleRow scores matmuls of tiles t>=1 at 0.5 cycles/row.
            # dim1=1 is memset to zero once (garbage would poison 0*NaN).
            QT_sb = [[apool.tile([128, 512], dt.bfloat16, name=f"QT{j}_{t}",
                                 tag=f"QT{j}_{t}") if t == 0 else None
                      for t in range(QT_TILES)] for j in range(2)]
            KT_sb = [[apool.tile([128, 512], dt.bfloat16, name=f"KT{j}_{t}",
                                 tag=f"KT{j}_{t}") if t == 0 else None
                      for t in range(QT_TILES)] for j in range(2)]
            QT8_sb = [[apool.tile([128, 2, 512], dt.float8e4,
                                  name=f"QT8{j}_{t}", tag=f"QT8{j}_{t}")
                       if t > 0 else None for t in range(QT_TILES)]
                      for j in range(2)]
            KT8_sb = [[apool.tile([128, 2, 512], dt.float8e4,
                                  name=f"KT8{j}_{t}", tag=f"KT8{j}_{t}")
                       for t in range(QT_TILES)] for j in range(2)]
            # dim1=1 zero-fills are emitted lazily at each group's start
            # (Pool would otherwise serialize 14 memsets ahead of the first
            # causal-mask affine_selects); tracked here to emit once.
            _ms_done = set()

            def memset_pad(tile8, key):
                if key not in _ms_done:
                    _ms_done.add(key)
                    nc.gpsimd.memset(tile8[:, 1, :], 0.0)
            # bf16 V only for key chunks 0-3 (tile-0's PV stays bf16: its
            # low-query rows average over too few keys to absorb fp8 noise)
            V_sb = [apool.tile([128, H_PER_CORE * VW], dt.bfloat16,
                               name=f"V{k}", tag=f"V{k}") for k in range(4)]
            # fp8 V in chunk-PAIR layout [128, 2(parity), 4*VW] for the
            # DoubleRow PV matmuls of tiles t>=1 (contract 256 keys per
            # instruction at 0.5 cycles/row: 4x the bf16 pair cost)
            V2_sb = [apool.tile([128, 2, H_PER_CORE * VWP], dt.float8e4,
                                name=f"V2{k}", tag=f"V2{k}")
                     for k in range(KCH // 2)]
            OT_sb = [[apool.tile([128, 512], dt.bfloat16, name=f"OT{j}_{t}",
                                 tag=f"OT{j}_{t}") for t in range(QT_TILES)]
                     for j in range(2)]

            qk_ps = {}

            def emit_qk_proj(w, j, t, c0=0, c1=CCH):
                # t=0: bf16, supports partial c-ranges with a persistent
                # psum group.  t>=1: fp8 DoubleRow -- 4 matmuls each
                # contracting a 256-row c-chunk-pair at 0.5 cycles/row.
                b_sb = (bqs_sb, bks_sb)[w]
                key = (w, j, t)
                if key not in qk_ps:
                    qk_ps[key] = mmpool.tile([128, 512], dt.float32,
                                             name="ps", tag="ps")
                ps = qk_ps[key]
                woff = w * DH + j * 128
                woff8 = 128 * (2 * j + w)
                if t == 0:
                    for c in range(c0, c1):
                        nc.tensor.matmul(
                            ps[:],
                            Wqk_sb[:, c, woff:woff + 128],
                            xT_t[t][:, c, :],
                            start=(c == 0), stop=(c == CCH - 1),
                            skip_group_check=True)
                    if c1 < CCH:
                        return
                else:
                    # c0/c1 are cp-pair indices here (0..4): partial ranges
                    # let the tile-1 prefix interleave Q and K chunk-pairs
                    # so both track the split x1 DMA arrivals
                    cp0, cp1 = (c0, min(c1, CCH // 2))
                    for cp in range(cp0, cp1):
                        nc.tensor.matmul(
                            ps[:],
                            Wqk8_sb[:, cp, :, woff8:woff8 + 128],
                            xT_t[t][:, cp, :, :],
                            start=(cp == 0), stop=(cp == CCH // 2 - 1),
                            perf_mode=mybir.MatmulPerfMode.DoubleRow,
                            skip_group_check=True)
                    if cp1 < CCH // 2:
                        return
                if w == 0:
                    d = QT_sb[j][t][:] if t == 0 else QT8_sb[j][t][:, 0, :]
                else:
                    d = KT_sb[j][t][:] if t == 0 else KT8_sb[j][t][:, 0, :]
                nc.vector.tensor_scalar_add(d, ps[:], b_sb[:, j:j + 1])
                del qk_ps[key]

            def emit_k03_f8(j):
                # fp8-projected K chunks 0-3, consumed only by tiles >= 1
                # (their rows average over >=512 keys, so the fp8 projection
                # noise is safe; tile 0 keeps its bf16 K)
                ps = mmpool.tile([128, 512], dt.float32, name="ps", tag="ps")
                woff8 = 128 * (2 * j + 1)
                for cp in range(CCH // 2):
                    nc.tensor.matmul(
                        ps[:],
                        Wqk8_sb[:, cp, :, woff8:woff8 + 128],
                        x08_sb[:, cp, :, :],
                        start=(cp == 0), stop=(cp == CCH // 2 - 1),
                        perf_mode=mybir.MatmulPerfMode.DoubleRow,
                        skip_group_check=True)
                nc.vector.tensor_scalar_add(KT8_sb[j][0][:, 0, :], ps[:],
                                            bks_sb[:, j:j + 1])

            def emit_v_proj(k):
                ps = mmpool.tile([128, DH], dt.float32, name="ps", tag="ps")
                if k < 4:
                    for c in range(CCH):
                        nc.tensor.matmul(
                            ps[:],
                            xT_t[0][:, c, (k % 4) * 128:(k % 4 + 1) * 128],
                            Wv_sb[:, c, :],
                            start=(c == 0), stop=(c == CCH - 1))
                else:
                    for cp in range(CCH // 2):
                        nc.tensor.matmul(
                            ps[:],
                            xT_t[k // 4][:, cp, :,
                                         (k % 4) * 128:(k % 4 + 1) * 128],
                            Wv8_sb[:, cp, :].rearrange(
                                "p (i e) -> p i e", i=2),
                            start=(cp == 0), stop=(cp == CCH // 2 - 1),
                            perf_mode=mybir.MatmulPerfMode.DoubleRow,
                            skip_group_check=True)
                dsts = [(V2_sb[k // 2][:, k % 2, :], VWP)]
                if k < 4:
                    dsts.append((V_sb[k][:], VW))
                for dst, vw in dsts:
                    v_dst = dst.rearrange("p (h e) -> p h e", e=vw)[:, :, 0:D_K]
                    nc.vector.tensor_tensor(
                        out=v_dst,
                        in0=ps[:].rearrange("p (h e) -> p h e", e=D_K),
                        in1=bvb_sb.rearrange("p (h e) -> p h e", e=D_K),
                        op=mybir.AluOpType.add)
                    od = dst.rearrange("p (h e) -> p h e", e=vw)[:, :, D_K]
                    nc.vector.tensor_scalar(
                        out=od, in0=bqkv_sb[:, 4:4 + H_PER_CORE], scalar1=0.0,
                        scalar2=1.0, op0=mybir.AluOpType.mult,
                        op1=mybir.AluOpType.add)

            # work queues drained into the exp-paced attention stages under
            # a per-stage PE-cost budget: projection groups for future tiles
            # first (they gate those tiles' attention), then output-projection
            # halves.  Tags order the force points: tile t start forces <= t,
            # the first PV flush of group (t, j0) forces <= t+0.5 (V tiles),
            # group (t, j1) start forces <= t+0.75 (its Q/K).
            pending_proj = []   # [tag, pe_cost_ns, fn, key]
            pending_oproj = []

            def force_item(key):
                # run (and remove) every queued item producing `key`, in
                # queue order (split projection groups share a key)
                for it in [x for x in pending_proj if x[3] == key]:
                    pending_proj.remove(it)
                    it[2]()

            def drain_budget(budget, sidx=99, allow_oproj=True,
                             on_act=False, reserve_oproj=0):
                spent = 0
                if budget <= 0:
                    return 0
                while True:
                    # first queued item whose DMA inputs have landed by
                    # this stage (draining earlier would head-of-line
                    # stall the in-order PE on the DMA semaphore)
                    it = next((x for x in pending_proj if x[4] <= sidx),
                              None)
                    if it is None or (spent and spent + it[1] > budget):
                        break
                    pending_proj.remove(it)
                    it[2]()
                    spent += it[1]
                while allow_oproj and len(pending_oproj) > reserve_oproj and spent + OPROJ_COST <= budget:
                    emit_oproj_half(*pending_oproj.pop(0), on_act=on_act)
                    spent += OPROJ_COST
                return spent

            def drain_all(on_act=False):
                while pending_proj:
                    pending_proj.pop(0)[2]()
                if TAIL_PAIR:
                    # pair the m-halves of each row into ONE dma (HWDGE
                    # costs 625ns per dma_start) with copies split ACT/DVE;
                    # alternate rows borrow the pv psum banks (free after
                    # the last normalize) to dodge mm-buf recycle stalls
                    row = 0
                    while len(pending_oproj) >= 2 and \
                            pending_oproj[0][1] == pending_oproj[1][1]:
                        (t_, tt, _) = pending_oproj.pop(0)
                        pending_oproj.pop(0)
                        row += 1
                        o_sb = opool.tile([128, 2, 512], dt.bfloat16,
                                          name="osb2", tag="osb2", bufs=4)
                        for m in range(2):
                            if row % 2:
                                ps = mmpool.tile([128, 512], dt.float32,
                                                 name="ps", tag="ps")
                            else:
                                ps = pvpool.tile([128, 512], dt.float32,
                                                 name=f"pv{m}",
                                                 tag=f"pv{m}", bufs=PV_BUFS)
                            for jj in range(2):
                                nc.tensor.matmul(
                                    ps[:],
                                    OT_sb[jj][t_][:, (tt % 4) * 128:
                                                  (tt % 4 + 1) * 128],
                                    WoT_sb[jj][:, m * 512:(m + 1) * 512],
                                    start=(jj == 0), stop=(jj == 1))
                            if m == 0:
                                nc.scalar.copy(o_sb[:, m, :], ps[:])
                            else:
                                nc.vector.tensor_copy(o_sb[:, m, :], ps[:])
                        nc.sync.dma_start(out[tt * 128:(tt + 1) * 128, :],
                                          o_sb[:])
                i = 0
                while pending_oproj:
                    emit_oproj_half(*pending_oproj.pop(0),
                                    on_act=(i % 2 == 0),
                                    use_sc=USC(i))
                    i += 1

            def emit_oproj_half(t, tt, m, on_act=False, use_sc=False):
                # one m-half (512 of 1024 output dims) of a 128-row slice of
                # the output projection: 2 matmuls + psum->sbuf copy + DMA.
                # Half-granularity gives the t3 drain pacing enough
                # resolution to keep every exp-paced stage PE-bound.
                o_sb = opool.tile([128, 512], dt.bfloat16, name="osb",
                                  tag="osb", bufs=OSB_BUFS)
                if use_sc:
                    # endgame: the sc psum pool is idle after the final exp;
                    # borrowing its banks doubles the psum tiles in flight so
                    # the tail matmuls stop waiting on copy-recycled mm bufs
                    ps = scpool.tile([128, 2, 512], dt.float32, name="sc",
                                     tag="sc")[:, 0, :]
                else:
                    ps = mmpool.tile([128, 512], dt.float32, name="ps",
                                     tag="ps")
                for j in range(2):
                    nc.tensor.matmul(
                        ps[:],
                        OT_sb[j][t][:, (tt % 4) * 128:(tt % 4 + 1) * 128],
                        WoT_sb[j][:, m * 512:(m + 1) * 512],
                        start=(j == 0), stop=(j == 1))
                # tail halves copy on ACT (idle after the last exp; DVE is
                # busy with the final normalize chain); halves drained
                # during attention stages copy on DVE (ACT is exp-bound)
                if on_act == "split":
                    # very last half: quarter-split the copy across ACT+DVE
                    # and DMA each quarter out as it lands, shortening the
                    # end-of-kernel copy->DMA->sem chain
                    nc.scalar.copy(o_sb[:, 0:256], ps[:, 0:256])
                    nc.sync.dma_start(
                        out[tt * 128:(tt + 1) * 128,
                            m * 512:m * 512 + 256], o_sb[:, 0:256])
                    nc.vector.tensor_copy(o_sb[:, 256:512], ps[:, 256:512])
                    nc.sync.dma_start(
                        out[tt * 128:(tt + 1) * 128,
                            m * 512 + 256:(m + 1) * 512], o_sb[:, 256:512])
                    return
                if on_act:
                    nc.scalar.copy(o_sb[:], ps[:])
                else:
                    nc.vector.tensor_copy(o_sb[:], ps[:])
                nc.sync.dma_start(
                    out[tt * 128:(tt + 1) * 128, m * 512:(m + 1) * 512],
                    o_sb[:])

            from functools import partial

            def emit_filler():
                for _ in range(N_FILLER):
                    fps = scpool.tile([128, 2, 512], dt.float32, name="sc",
                                      tag="sc")
                    nc.tensor.matmul(fps[:, 0, :], warm_sb[:, 0:128],
                                     warm_sb[:], start=True, stop=True,
                                     skip_group_check=True)

            # global projection work queue, ordered by first use under the
            # [1, 2, 0, 3] tile processing order.  Keys let the attention
            # loop force exactly the group a pair is about to consume.
            TO = [int(c) for c in os.environ.get("K_TO", "1203")]

            def q(tag, cost, fn, key, ready=0):
                pending_proj.append([tag, cost, fn, key, ready])

            # tile-1 j0 prefix, emitted directly in cp-interleaved order:
            # Q/K chunk-pairs 0-1 run on the first x1 half-DMA while
            # chunk-pairs 2-3 wait for the second
            emit_qk_proj(0, 0, 1, 0, 2)
            emit_qk_proj(1, 0, 1, 0, 2)
            emit_qk_proj(0, 0, 1, 2, 4)
            emit_qk_proj(1, 0, 1, 2, 4)
            q(1, 427, partial(emit_k03_f8, 0), ("K8", 0, 0), 1)
            q(1, 427, partial(emit_k03_f8, 1), ("K8", 1, 0), 1)
            q(1, 427, partial(emit_qk_proj, 0, 1, 1), ("Q", 1, 1), 1)
            q(1, 427, partial(emit_qk_proj, 1, 1, 1), ("K", 1, 1), 1)
            for k in (4, 5, 6, 7):
                q(1, 250, partial(emit_v_proj, k), ("V", k), 2)
            for k in (0, 1, 2, 3):
                q(1, 900, partial(emit_v_proj, k), ("V", k), 3)
            q(2, 427, partial(emit_qk_proj, 0, 0, 2), ("Q", 0, 2), 5)
            q(2, 427, partial(emit_qk_proj, 1, 0, 2), ("K", 0, 2), 5)
            for k in (8, 9, 10, 11):
                q(2, 250, partial(emit_v_proj, k), ("V", k), 5)
            q(2, 427, partial(emit_qk_proj, 0, 1, 2), ("Q", 1, 2), 5)
            q(2, 427, partial(emit_qk_proj, 1, 1, 2), ("K", 1, 2), 5)
            q(3, 853, partial(emit_qk_proj, 1, 0, 0, 0, 4), ("K", 0, 0), 5)
            q(3, 900, partial(emit_qk_proj, 1, 0, 0, 4, 8), ("K", 0, 0), 5)
            q(3, 853, partial(emit_qk_proj, 0, 0, 0, 0, 4), ("Q", 0, 0), 5)
            q(3, 900, partial(emit_qk_proj, 0, 0, 0, 4, 8), ("Q", 0, 0), 5)
            q(3, 853, partial(emit_qk_proj, 1, 1, 0, 0, 4), ("K", 1, 0), 5)
            q(3, 900, partial(emit_qk_proj, 1, 1, 0, 4, 8), ("K", 1, 0), 5)
            q(3, 853, partial(emit_qk_proj, 0, 1, 0, 0, 4), ("Q", 1, 0), 5)
            q(3, 900, partial(emit_qk_proj, 0, 1, 0, 4, 8), ("Q", 1, 0), 5)
            q(4, 427, partial(emit_qk_proj, 0, 0, 3), ("Q", 0, 3), 6)
            q(4, 427, partial(emit_qk_proj, 1, 0, 3), ("K", 0, 3), 6)
            for k in (12, 13, 14, 15):
                q(4, 250, partial(emit_v_proj, k), ("V", k), 6)
            q(4, 427, partial(emit_qk_proj, 0, 1, 3), ("Q", 1, 3), 6)
            q(4, 427, partial(emit_qk_proj, 1, 1, 3), ("K", 1, 3), 6)

            for j in range(2):
                nc.sync.dma_start(WoT_sb[j][:],
                                  WoT[j * 128:(j + 1) * 128, :])
            def flush(mms_pp, final):
                for p in range(2):
                    for i, (lhsT, rhs, o, st, pm) in enumerate(mms_pp[p]):
                        nc.tensor.matmul(
                            o, lhsT, rhs, start=st,
                            stop=(final and i == len(mms_pp[p]) - 1),
                            perf_mode=pm, skip_group_check=True)

            pending_norm = []

            def normalize_group(t, j, pvs):
                # OT = PV * (1/denom).  DVE tensor_tensor can read only ONE
                # psum operand, so the broadcast goes through gpsimd to
                # SBUF.  The mults are DEFERRED one stage: an in-order DVE
                # would otherwise head-of-line stall on the Pool broadcast
                # roundtrip, delaying the next group's Q/K bias write.
                bcs = []
                for p in range(2):
                    rc = smpool.tile([1, 512], dt.float32, name="rc",
                                     tag="rc", bufs=SM_BUFS)
                    bc = smpool.tile([64, 512], dt.float32, name="bc",
                                     tag="bc", bufs=SM_BUFS)
                    nc.vector.reciprocal(rc[:], pvs[p][D_K:VW, :])
                    nc.gpsimd.partition_broadcast(bc[:], rc[:])
                    bcs.append(bc)

                def mults():
                    for p in range(2):
                        nc.vector.tensor_tensor(
                            out=OT_sb[j][t][p * 64:(p + 1) * 64, :],
                            in0=pvs[p][0:D_K, :], in1=bcs[p][:],
                            op=mybir.AluOpType.mult)
                pending_norm.append(mults)

            def normalize_last(t, j, pvs):
                # final group: the first 128 columns get their own short
                # recip/bcast/mult so the first tail oproj half (which
                # reads OT cols 0:128) unblocks as early as possible
                rcs = [smpool.tile([1, 512], dt.float32, name="rc",
                                   tag="rc", bufs=SM_BUFS) for _ in range(2)]
                bcs = [smpool.tile([64, 512], dt.float32, name="bc",
                                   tag="bc", bufs=SM_BUFS) for _ in range(2)]
                for hh in range(2):
                    s = slice(hh * 256, (hh + 1) * 256)
                    for p in range(2):
                        nc.vector.reciprocal(rcs[p][:, s], pvs[p][D_K:VW, s])
                        nc.gpsimd.partition_broadcast(bcs[p][:, s],
                                                      rcs[p][:, s])
                    for cc in (2 * hh, 2 * hh + 1):
                        sc_ = slice(cc * 128, (cc + 1) * 128)
                        for p in range(2):
                            nc.vector.tensor_tensor(
                                out=OT_sb[j][t][p * 64:(p + 1) * 64, sc_],
                                in0=pvs[p][0:D_K, sc_], in1=bcs[p][:, sc_],
                                op=mybir.AluOpType.mult)

            # ---- global stage pipeline: one stage per (tile, head-pair,
            # key-chunk-pair).  The PV-flush software pipeline runs ACROSS
            # group and tile boundaries, so a group's flush+normalize tail
            # overlaps the next group's scores/exp instead of serializing
            # at each boundary.  Diagonal pairs first within each group.
            stages = []
            for t in TO:
                for j in range(2):
                    pis = [2 * t, 2 * t + 1] + list(range(2 * t))
                    for pii, pi in enumerate(pis):
                        stages.append((t, j, pi, pii, len(pis)))

            pend = []          # (t, j, pi, mms_pp, pvs, final)
            group_pvs = {}
            LAST = (TO[-1], 1)

            def pop_flush():
                t_, j_, pi_, mms_, pvs_, final_ = pend.pop(0)
                force_item(("V", 2 * pi_))
                force_item(("V", 2 * pi_ + 1))
                flush(mms_, final_)
                if final_:
                    if (t_, j_) == LAST:
                        normalize_last(t_, j_, pvs_)
                    else:
                        normalize_group(t_, j_, pvs_)
                    if j_ == 1:
                        # both head-pairs' OT ready: queue the tile's
                        # output-projection halves
                        for tt in range(4 * t_, 4 * t_ + 4):
                            for m in range(2):
                                pending_oproj.append((t_, tt, m))

            for sidx, (t, j, pi, pii, npair) in enumerate(stages):
                if pii == 0:
                    if (t, j) == (TO[0], 0):
                        # head-critical zero-fills: tile 1's own k-tile
                        # padding plus the chunk-0-3 K it reads
                        for jj in range(2):
                            memset_pad(QT8_sb[jj][1], ("q", jj, 1))
                            memset_pad(KT8_sb[jj][1], ("k", jj, 1))
                            memset_pad(KT8_sb[jj][0], ("k", jj, 0))
                    elif j == 1 and t in (TO[0], TO[1]):
                        # prefetch the NEXT tile's pads a whole group early
                        # so Pool has slack to run them behind the masks
                        tn = TO[TO.index(t) + 1]
                        for jj in range(2):
                            if tn > 0:
                                memset_pad(QT8_sb[jj][tn], ("q", jj, tn))
                            memset_pad(KT8_sb[jj][tn], ("k", jj, tn))
                    elif j == 0 and t == TO[2]:
                        for jj in range(2):
                            memset_pad(QT8_sb[jj][3], ("q", jj, 3))
                            memset_pad(KT8_sb[jj][3], ("k", jj, 3))
                    # this group's own Q/K must exist before its scores
                    force_item(("Q", j, t))
                    force_item(("K", j, t))
                    group_pvs[(t, j)] = [
                        pvpool.tile([128, 512], dt.float32, name=f"pv{p}",
                                    tag=f"pv{p}", bufs=PV_BUFS)
                        for p in range(2)]
                pvs = group_pvs[(t, j)]
                diag = pi >= 2 * t
                first_pair = pii == 0
                # for the second diagonal pair (r0=2) only columns
                # q >= 128*r0 can be unmasked for either half, so the
                # scores matmuls and exp skip the dead columns.
                q0 = 128 * 2 * (pi - 2 * t) if diag else 0
                use8 = t > 0  # fp8 DoubleRow scores+PV for tiles >= 1
                pt_dt = dt.float8e4 if use8 else dt.bfloat16
                pt_pool = pt8pool if use8 else ptpool
                # the K projection tile this pair's scores read
                if use8 and pi // 2 == 0:
                    force_item(("K8", j, 0))
                else:
                    force_item(("K", j, pi // 2))
                pts = []
                for p in range(2):
                    lo = p * 64
                    sc = scpool.tile([128, 2, 512], dt.float32,
                                     name="sc", tag="sc")
                    for half in range(2):
                        c = 2 * pi + half
                        # per-half trim: k-block r = 2(pi-2t)+half is only
                        # unmasked for q >= 128*r
                        q0h = 128 * (2 * (pi - 2 * t) + half) \
                            if diag else 0
                        if use8:
                            nc.tensor.matmul(
                                sc[:, half, q0h:],
                                KT8_sb[j][(2 * pi) // 4][
                                    lo:lo + 64, :,
                                    (c % 4) * 128:(c % 4 + 1) * 128],
                                QT8_sb[j][t][lo:lo + 64, :, q0h:],
                                start=True, stop=True,
                                tile_position=(lo, 0),
                                perf_mode=mybir.MatmulPerfMode.DoubleRow)
                        else:
                            nc.tensor.matmul(
                                sc[:, half, q0h:],
                                KT_sb[j][(2 * pi) // 4][
                                    lo:lo + 64,
                                    (c % 4) * 128:(c % 4 + 1) * 128],
                                QT_sb[j][t][lo:lo + 64, q0h:],
                                start=True, stop=True,
                                tile_position=(lo, 0))
                    pt = pt_pool.tile([128, 2, 512], pt_dt,
                                      name="pt", tag="pt")
                    nc.scalar.activation(pt[:, :, q0:],
                                         sc[:, :, q0:], AF.Exp)
                    pts.append(pt)
                mms_pp = []
                for p in range(2):
                    pt = pts[p]
                    h = 2 * j + p
                    vsl = lambda c: V_sb[c][:, h * VW:(h + 1) * VW]
                    mms = []  # (lhsT, rhs, out, start, perf_mode)
                    if use8:
                        v2 = V2_sb[pi][:, :, h * VWP:h * VWP + VW]
                        if diag:
                            r0 = 2 * (pi - 2 * t)
                            # half0: zero below-diagonal of its 128x128
                            # triangle block
                            tri = pt[:, 0, 128 * r0:128 * (r0 + 1)]
                            nc.gpsimd.affine_select(
                                out=tri, in_=tri,
                                compare_op=mybir.AluOpType.is_ge,
                                fill=0.0, base=0, pattern=[[1, 128]],
                                channel_multiplier=-1)
                            # half1: cols [128r0, 128(r0+1)) are entirely
                            # below-diagonal plus its own triangle block --
                            # one widened select covers both
                            tri2 = pt[:, 1, 128 * r0:128 * (r0 + 2)]
                            nc.gpsimd.affine_select(
                                out=tri2, in_=tri2,
                                compare_op=mybir.AluOpType.is_ge,
                                fill=0.0, base=-128, pattern=[[1, 256]],
                                channel_multiplier=-1)
                            mms.append((v2, pt[:, :, 128 * r0:],
                                        pvs[p][0:VW, 128 * r0:],
                                        first_pair,
                                        mybir.MatmulPerfMode.DoubleRow))
                        else:
                            mms.append((v2, pt[:, :, :],
                                        pvs[p][0:VW, :], first_pair,
                                        mybir.MatmulPerfMode.DoubleRow))
                    elif not diag:
                        for half in range(2):
                            mms.append((vsl(2 * pi + half),
                                        pt[:, half, :], pvs[p][0:VW, :],
                                        first_pair and half == 0, None))
                    else:
                        r0 = 2 * (pi - 2 * t)
                        for half in range(2):
                            r = r0 + half
                            tri = pt[:, half, 128 * r:128 * (r + 1)]
                            nc.gpsimd.affine_select(
                                out=tri, in_=tri,
                                compare_op=mybir.AluOpType.is_ge,
                                fill=0.0, base=0, pattern=[[1, 128]],
                                channel_multiplier=-1)
                        for half in range(2):
                            r = r0 + half
                            mms.append((vsl(2 * pi + half),
                                        pt[:, half, 128 * r:],
                                        pvs[p][0:VW, 128 * r:],
                                        first_pair and half == 0, None))
                    mms_pp.append(mms)
                pend.append((t, j, pi, mms_pp, pvs, pii == npair - 1))
                while pending_norm:
                    pending_norm.pop(0)()
                drain_budget(BUD_T0 if t == 0 else BUD, sidx=sidx,
                             reserve_oproj=RESERVE)
                while len(pend) > PV_DEPTH:
                    pop_flush()
            # pipeline tail: flush the last pairs, then the reserved oproj
            # halves fill the final normalize window
            while pend:
                pop_flush()
                while pending_norm:
                    pending_norm.pop(0)()
            drain_all()
            if TAIL_PAIR:
                # tail: one o_sb + one DMA per tt row-block (the 8 tail
                # DMAs otherwise serialize on the 625ns-per-DMA HWDGE)
                for i in range(0, len(pending_oproj), 2):
                    (t_, tt, _), _ = pending_oproj[i], pending_oproj[i + 1]
                    o_sb = opool.tile([128, 2, 512], dt.bfloat16,
                                      name="osb2", tag="osb2", bufs=2)
                    for m in range(2):
                        ps = mmpool.tile([128, 512], dt.float32, name="ps",
                                         tag="ps")
                        for j in range(2):
                            nc.tensor.matmul(
                                ps[:],
                                OT_sb[j][t_][:,
                                             (tt % 4) * 128:(tt % 4 + 1) * 128],
                                WoT_sb[j][:, m * 512:(m + 1) * 512],
                                start=(j == 0), stop=(j == 1))
                        if m == 0:
                            nc.scalar.copy(o_sb[:, m, :], ps[:])
                        else:
                            nc.vector.tensor_copy(o_sb[:, m, :], ps[:])
                    nc.sync.dma_start(out[tt * 128:(tt + 1) * 128, :],
                                      o_sb[:])
            else:
                for i, args in enumerate(pending_oproj):
                    last = i == len(pending_oproj) - 1
                    emit_oproj_half(*args,
                                    on_act="split" if (last and SPLIT_LAST)
                                    else OACT(i),
                                    use_sc=USC(i))
    nc.compile()
    return nc


def _in_maps(x, Wq, bq, Wk, bk, Wv, bv, Wo, bo):
    maps = []
    F8 = ml_dtypes.float8_e4m3fn
    dts = {"xT0": BF16, "xT8": F8, "x08T": F8, "WqkT": BF16, "Wqk8T": F8,
           "WvT": BF16, "Wv8T": F8, "WoT": BF16, "bqkv": np.float32}
    for core in range(N_CORES):
        b = core // 4
        h0 = (core % 4) * H_PER_CORE
        hs = slice(h0 * D_K, (h0 + H_PER_CORE) * D_K)
        xTb = np.ascontiguousarray(x[b].T)
        wqk = np.concatenate([
            np.ascontiguousarray(Wq[hs, :].T) * 0.125,
            np.ascontiguousarray(Wk[hs, :].T)], axis=1)
        wv = np.ascontiguousarray(Wv[hs, :].T)
        m = {
            "xT0": xTb[:, 0:512],
            "xT8": xTb[:, 512:],
            "x08T": xTb[:, 0:512],
            "WqkT": wqk,
            "Wqk8T": np.concatenate([wqk[:, 0:128], wqk[:, 256:384],
                                     wqk[:, 128:256], wqk[:, 384:512]],
                                    axis=1),
            "WvT": wv,
            "Wv8T": wv.reshape(CCH // 2, 2, 128, DH).transpose(
                0, 2, 1, 3).reshape(D_MODEL // 2, 2 * DH),
            "WoT": np.ascontiguousarray(Wo[:, hs].T),
            "bqkv": np.concatenate([
                np.ascontiguousarray((bq[hs] * 0.125).reshape(2, 128).T),
                np.ascontiguousarray(bk[hs].reshape(2, 128).T),
                np.broadcast_to(bv[hs], (128, DH)),
            ], axis=1),
        }
        maps.append({k: np.ascontiguousarray(v, dtype=dts[k])
                     for k, v in m.items()})
    return maps


def kernel(x, Wq, bq, Wk, bk, Wv, bv, Wo, bo, _trace=False):
    if "nc" not in _CACHE:
        _CACHE["nc"] = _build()
    nc = _CACHE["nc"]
    in_maps = _in_maps(np.asarray(x, dtype=np.float32),
                       np.asarray(Wq, dtype=np.float32),
                       np.asarray(bq, dtype=np.float32),
                       np.asarray(Wk, dtype=np.float32),
                       np.asarray(bk, dtype=np.float32),
                       np.asarray(Wv, dtype=np.float32),
                       np.asarray(bv, dtype=np.float32),
                       np.asarray(Wo, dtype=np.float32),
                       np.asarray(bo, dtype=np.float32))
    res = run_bass_kernel_spmd(nc, in_maps, core_ids=list(range(N_CORES)),
                               trace=_trace)
    bo = np.asarray(bo, dtype=np.float32)
    out = np.zeros((B, S, D_MODEL), dtype=np.float32)
    for b in range(B):
        acc = res.results[b * 4]["out"].astype(np.float64)
        for core in range(b * 4 + 1, b * 4 + 4):
            acc = acc + res.results[core]["out"]
        out[b] = (acc + bo).astype(np.float32)
    if _trace:
        return out, res
    return out



# revision 63
# speedup vs baseline: 1.0258x; 1.0258x over previous
"""MultiHeadAttention (B=2, S=2048, d_model=1024, 16 heads, causal) on 8 TRN2 cores.

Sharding: core i handles batch (i//4) and heads 4*(i%4) .. 4*(i%4)+4 (tensor
parallel over heads within a batch).  Each core computes its 4 heads'
Q/K/V projections, causal attention, and the partial output projection
(contribution of its 256 head-dims to all 1024 output dims).  The host sums
the 4 bf16 partials per batch in float64 and adds the output bias.

Precision strategy (validated against the fp32 reference in numpy; the
harness gate is scale-relative absmax 2e-2, this kernel lands ~9.1e-3):
  - everything touching QUERIES < 512 (q-tile 0) runs bf16: those rows
    average over too few keys to absorb fp8 quantization noise (q=0 outputs
    V[0] exactly, so fp8's ~6% element error would land raw in the output).
  - tiles 1-3 run fp8e4m3 end to end: x tiles 1-3 (plus an fp8 copy of
    x tile 0 that feeds the fp8 K chunks 0-3 those tiles read, so their
    early pairs never wait on the 1MB bf16 x0 transfer) and the
    Wq/Wk/Wv copies ship fp8 from the host; Q/K/V projections, QK^T scores, and the PV
    matmul all use fp8 DoubleRow perf mode, which the PE charges at 0.5
    cycles/row while contracting TWO 128-row k-tiles per instruction:
      * projections: c-chunk pairs packed -> 4x cheaper than bf16
      * PV: two key-chunks packed ([128, 2(parity), 4*68] fp8 V layout,
        head stride padded to 68 for the 16B outer-stride ISA rule) -> 4x
      * scores: d_k=64 only fills half the array, so the second k-tile is
        ZERO-padded (KT8/QT8 are [128, 2, 512] with dim1=1 memset once;
        0 * garbage would be NaN-poisoned otherwise) -> 2x
    Errors average out over >=512 keys: the fp8 kernel's absmax error is
    within 2.5x of the all-bf16 kernel's.
  - the output projection stays bf16 (fp8 fails the gate: contraction 256
    gives no averaging), PSUM accumulation is fp32 throughout, exp output
    is written directly as fp8 for tiles >= 1 (free on ACT).
  - bias handling: 1/sqrt(d_k) folds into Wq/bq on the host; V's bias rides
    a broadcast row of the bqkv tensor; the softmax denominator comes from
    a ones-column appended to V (accumulated by the same PV matmuls).

The machine balance after fp8: ACT (exp) ~77.5us busy is the bottleneck
(PE ~59, DVE ~60, Pool ~43), so the whole kernel is scheduled as ONE
global pipeline of 40 exp-paced stages (one per (tile, head-pair,
key-chunk-pair)):
  - tiles process in order [1, 2, 0, 3]: the j0 half of the fp8 Wqk
    plus fp8 x1 is all the first exp needs, so it lands first and the
    exp stream starts at ~7.4us (vs ~15us for the original bf16-first
    supply).  The DMA chain is sequenced by strict NEED time -- all fp8
    supply (x08, Wqk8B, Wv8, x82, x83) ships before the bf16 tile-0
    supply (Wv, x0, Wqk columns), which nothing reads before stage ~20.
    The whole exp stream is anchored to DMA landing times, so every ns
    squeezed out of the chain moves the stream end 1:1.  fp8 tensors are
    host-packed (two c-rows per DRAM row) so both DMA sides keep >=512B
    contiguous runs (256B runs pay a 2x descriptor-latency penalty).
  - PV flushes run a 7-deep software pipeline ACROSS group and tile
    boundaries: a group's flush + normalize tail overlaps the next
    group's scores/exp instead of serializing at each boundary.  A
    group's normalize MULTS are deferred one stage so the in-order DVE
    never head-of-line stalls on the Pool broadcast roundtrip.
  - projection/output-projection work drains into each stage under a PE
    budget (~1000ns) with per-item DMA-readiness stages (draining a
    not-yet-landed item would head-of-line stall the in-order PE), and
    keyed force points guarantee a group's Q/K/V exist exactly when its
    scores or flushes consume them.
  - causal masking: per-half column trims on the diagonal pairs plus
    gpsimd affine_selects on the two 128x128 triangle blocks (the fp8
    path widens one select to also zero the below-diagonal block that
    column ranges used to exclude).
  - the endgame pairs output rows into single DMAs (HWDGE costs 625ns
    per dma_start) EXCEPT the final row, which goes as two half DMAs so
    each issues the moment its psum->sbuf copy lands; copies split
    ACT/DVE, psum alternates between the mm and then-idle pv pools, and
    the last normalize is column-split so the tail starts early.  No
    oproj reserve: the tail queue must stay row-aligned or the pair
    merges all fall through to issue-chain-gated single DMAs.

Cost-model (TimelineSim) estimate: 98.42 us/core (fp32r baseline kernel:
146.0 us; bf16 predecessor: 130.6 us).  Scale-relative absmax error vs
the fp32 reference: 9.29e-3 (gate is 2e-2).
"""

import numpy as np
import ml_dtypes

import concourse.bass as bass
import concourse.tile as tile
import concourse.mybir as mybir
from concourse import bacc
from concourse.bass_utils import run_bass_kernel_spmd

dt = mybir.dt
AF = mybir.ActivationFunctionType
BF16 = ml_dtypes.bfloat16

D_MODEL = 1024
N_HEADS = 16
D_K = 64
B = 2
S = 2048
H_PER_CORE = 4
DH = H_PER_CORE * D_K  # 256
N_CORES = 8
CCH = D_MODEL // 128  # 8 contraction chunks
QT_TILES = S // 512  # 4
KCH = S // 128  # 16 key chunks
VW = D_K + 1  # 65
VWP = 68  # fp8 V per-head stride: DoubleRow needs 16B-aligned outer strides

_CACHE = {}

import os

CEXP = int(os.environ.get("K_CEXP", "0"))
SC_BUFS = int(os.environ.get("K_SC_BUFS", "2"))
MM_BUFS = 2
PV_BUFS = 1
PT_BUFS = int(os.environ.get("K_PT", "13"))
N_WARMUP = int(os.environ.get("K_WARMUP", "5"))
WARM_MEMSET = int(os.environ.get("K_WARM_MEMSET", "1"))
DRAIN2 = int(os.environ.get("K_DRAIN2", "0"))
FLUSH1 = int(os.environ.get("K_FLUSH1", "0"))
XPRE = int(os.environ.get("K_XPRE", "2"))
RESERVE = int(os.environ.get("K_RESERVE", "0"))
GDRAIN = int(os.environ.get("K_GDRAIN", "3"))
BUD_T0 = int(os.environ.get("K_BUD_T0", "700"))
BUD = int(os.environ.get("K_BUD", "1000"))
BUD_G = int(os.environ.get("K_BUD_G", "1300"))
OPROJ_COST = int(os.environ.get("K_OPROJ_COST", "480"))
N_FILLER = int(os.environ.get("K_FILLER", "0"))
NORM_ORDER = os.environ.get("K_NORM", "paired")
SPLIT_LAST = int(os.environ.get("K_SPLIT_LAST", "0"))
TAIL_PAIR = int(os.environ.get("K_TAIL_PAIR", "1"))
PACE_R = int(os.environ.get("K_PACE_R", "0"))
PV_DEPTH = int(os.environ.get("K_PV_DEPTH", "7"))
PV_LAST = int(os.environ.get("K_PV_LAST", "4"))
WV_SPLIT = int(os.environ.get("K_WV_SPLIT", "1"))
WV_EARLY = int(os.environ.get("K_WV_EARLY", "0"))
BQKV_EARLY = int(os.environ.get("K_BQKV_EARLY", "0"))
OSB_BUFS = int(os.environ.get("K_OSB", "6"))
SM_BUFS = int(os.environ.get("K_SM", "8"))
_oact = os.environ.get("K_OACT", "all")
OACT = {"all": lambda i: True, "mix": lambda i: i < 3 or i % 2 == 0,
        "alt": lambda i: i % 2 == 0}[_oact]
_usc = os.environ.get("K_USC", "none")
USC = {"none": lambda i: False, "alt": lambda i: i % 2 == 1,
       "all": lambda i: True}[_usc]


def _build():
    nc = bacc.Bacc("TRN2", target_bir_lowering=False, debug=False,
                   num_devices=N_CORES)

    # x tile 0 ships bf16 (tile-0 numerics need it); tiles 1-3 ship fp8.
    # Weights ship in both precisions: bf16 for the tile-0/chunk-0-3
    # projections, fp8 for the DoubleRow projections of everything else.
    xT0 = nc.dram_tensor("xT0", [D_MODEL, 512], dt.bfloat16,
                         kind="ExternalInput").ap()
    xT8 = nc.dram_tensor("xT8", [D_MODEL, S - 512], dt.float8e4,
                         kind="ExternalInput").ap()
    x08T = nc.dram_tensor("x08T", [D_MODEL, 512], dt.float8e4,
                          kind="ExternalInput").ap()
    WqkT = nc.dram_tensor("WqkT", [D_MODEL, 2 * DH], dt.bfloat16,
                          kind="ExternalInput").ap()
    Wqk8AT = nc.dram_tensor("Wqk8AT", [D_MODEL // 2, 2 * DH], dt.float8e4,
                            kind="ExternalInput").ap()
    Wqk8BT = nc.dram_tensor("Wqk8BT", [D_MODEL // 2, 2 * DH], dt.float8e4,
                            kind="ExternalInput").ap()
    WvT = nc.dram_tensor("WvT", [D_MODEL, DH], dt.bfloat16, kind="ExternalInput").ap()
    Wv8T = nc.dram_tensor("Wv8T", [D_MODEL // 2, 2 * DH], dt.float8e4,
                          kind="ExternalInput").ap()
    WoT = nc.dram_tensor("WoT", [DH, D_MODEL], dt.bfloat16, kind="ExternalInput").ap()
    bqkv = nc.dram_tensor("bqkv", [128, 4 + DH], dt.float32,
                          kind="ExternalInput").ap()
    out = nc.dram_tensor("out", [S, D_MODEL], dt.bfloat16, kind="ExternalOutput").ap()


    with tile.TileContext(nc) as tc:
        with (
            tc.tile_pool(name="weights", bufs=1) as wpool,
            tc.tile_pool(name="acts", bufs=1) as apool,
            tc.tile_pool(name="pt", bufs=PT_BUFS) as ptpool,
            tc.tile_pool(name="pt8", bufs=PT_BUFS) as pt8pool,
            tc.tile_pool(name="sm", bufs=4) as smpool,
            tc.tile_pool(name="outsb", bufs=4) as opool,
            tc.tile_pool(name="mm", bufs=MM_BUFS, space="PSUM") as mmpool,
            tc.tile_pool(name="sc", bufs=SC_BUFS, space="PSUM") as scpool,
            tc.tile_pool(name="pv", bufs=1, space="PSUM") as pvpool,
        ):
            # ---- PE warmup: the cost model's p-state ramp needs ~3us of
            # continuous PE activity before matmuls run at full clock.  The
            # first input DMAs take ~1-2us to land, so burn that window with
            # dummy matmuls on a scratch tile (never read back); real matmuls
            # queue behind them and start fully warm.
            warm_sb = wpool.tile([128, 512], dt.bfloat16, name="warm",
                                 tag="warm")
            if WARM_MEMSET:
                nc.vector.memset(warm_sb[:], 0.0)
            warm_ps = mmpool.tile([128, 512], dt.float32, name="ps", tag="ps")
            for wi in range(N_WARMUP):
                nc.tensor.matmul(warm_ps[:], warm_sb[:, 0:128], warm_sb[:],
                                 start=True, stop=True, skip_group_check=True)

            # ---- input loads, ordered by first use under the [1,2,0,3]
            # tile processing order: the fp8 Wqk + fp8 x1 land first (~3us)
            # so tile 1's DoubleRow projections and first exp start early;
            # the bf16 Wqk/x0 (tile-0 path) stream in behind them.
            # tile0 x: [128, c(8), 512] bf16; tiles 1-3: [128, cp(4),
            # 2(c-parity), 512] fp8 -- the DoubleRow k-tile pair layout.
            xT_t = [wpool.tile([128, CCH, 512], dt.bfloat16, name="xTt0",
                               tag="xTt0") if t == 0 else
                    wpool.tile([128, CCH // 2, 2, 512], dt.float8e4,
                               name=f"xTt{t}", tag=f"xTt{t}")
                    for t in range(QT_TILES)]
            xT0_r = xT0.rearrange("(c p) q -> p c q", p=128)
            x8_r = xT8.rearrange("(cp i p) q -> p cp i q", p=128, i=2)
            x08_r = x08T.rearrange("(cp i p) q -> p cp i q", p=128, i=2)
            x08_sb = wpool.tile([128, CCH // 2, 2, 512], dt.float8e4,
                                name="x08", tag="x08")
            Wqk_sb = wpool.tile([128, CCH, 2 * DH], dt.bfloat16)
            # fp8 Wqk ships as TWO host-packed half tensors (j0 = [Q|K]
    # cols of head-pair 0, j1 likewise), each [512, 512] with two
            # c-rows per DRAM row so both DMA sides keep 512B runs; the
            # j0 half plus fp8 x1 is all the first exp needs
            Wqk8_sb = [wpool.tile([128, CCH // 2, 2 * DH], dt.float8e4,
                                  name=f"W8{j}", tag=f"W8{j}")
                       for j in range(2)]
            Wv_sb = wpool.tile([128, CCH, DH], dt.bfloat16)
            # fp8 Wv ships host-packed as [512, 2*DH] (two c-rows per DRAM
            # row) so both DMA sides have 512B contiguous runs -- 256B runs
            # pay a 2x DMA latency penalty
            Wv8_sb = wpool.tile([128, CCH // 2, 2 * DH], dt.float8e4)
            wqk_r = WqkT.rearrange("(c p) d -> p c d", p=128)
            wqk8a_r = Wqk8AT.rearrange("(cp p) e -> p cp e", p=128)
            wqk8b_r = Wqk8BT.rearrange("(cp p) e -> p cp e", p=128)
            wv_r = WvT.rearrange("(c p) d -> p c d", p=128)
            wv8_r = Wv8T.rearrange("(cp p) e -> p cp e", p=128)
            bqkv_sb = wpool.tile([128, 4 + DH], dt.float32)
            # head-critical: the j0 half of the fp8 Wqk (host layout is
            # [Qj0|Kj0|Qj1|Kj1] so it is contiguous), then fp8 x1 -- these
            # gate tile 1's first scores.  bqkv (bias, needed by the first
            # DVE write) follows, then the bf16 x0 (gates the chunk-0-3
            # K/V projections tile 1's later pairs read).
            nc.sync.dma_start(Wqk8_sb[0][:], wqk8a_r)
            nc.sync.dma_start(xT_t[1][:, 0:2, :, :], x8_r[:, 0:2, :, 0:512])
            nc.sync.dma_start(xT_t[1][:, 2:4, :, :], x8_r[:, 2:4, :, 0:512])
            nc.sync.dma_start(bqkv_sb[:], bqkv)
            # fp8 copy of x tile 0: feeds the fp8 K chunks 0-3 that tiles
            # >=1 read, so their early pairs never wait on the 1MB bf16 x0
            nc.sync.dma_start(x08_sb[:], x08_r)
            nc.sync.dma_start(Wqk8_sb[1][:], wqk8b_r)
            nc.sync.dma_start(Wv8_sb[:], wv8_r)
            bqs_sb = bqkv_sb[:, 0:2]
            bks_sb = bqkv_sb[:, 2:4]
            bvb_sb = bqkv_sb[:, 4:4 + DH]
            WoT_sb = [wpool.tile([128, D_MODEL], dt.bfloat16, name=f"Wo{j}",
                                 tag=f"Wo{j}") for j in range(2)]
            # remaining fp8 x tiles next (tiles 2/3 start at stages 8/24),
            # THEN the bf16 tile-0 supply -- t0 runs third, so nothing
            # needs it before ~stage 20
            for tx in (2, 3):
                nc.sync.dma_start(
                    xT_t[tx][:],
                    x8_r[:, :, :, (tx - 1) * 512:tx * 512])
            nc.sync.dma_start(Wv_sb[:, 0:4, :], wv_r[:, 0:4, :])
            nc.sync.dma_start(Wv_sb[:, 4:8, :], wv_r[:, 4:8, :])
            nc.sync.dma_start(xT_t[0][:, 0:4, :], xT0_r[:, 0:4, :])
            nc.sync.dma_start(xT_t[0][:, 4:8, :], xT0_r[:, 4:8, :])
            nc.sync.dma_start(Wqk_sb[:, :, 256:512], wqk_r[:, :, 256:512])
            nc.sync.dma_start(Wqk_sb[:, :, 0:256], wqk_r[:, :, 0:256])

            # ---- fully streamed per q-tile: projections for tile t, then
            # attention for tile t (overlaps next tile's projections on PE),
            # then tile t's slice of the output projection.
            # bf16 Q/K only for tile 0 (its low-query rows need bf16
            # scores); fp8 zero-padded [128, 2(k-tile), 512] Q/K feed the
            # DoubleRow scores matmuls of tiles t>=1 at 0.5 cycles/row.
            # dim1=1 is memset to zero once (garbage would poison 0*NaN).
            QT_sb = [[apool.tile([128, 512], dt.bfloat16, name=f"QT{j}_{t}",
                                 tag=f"QT{j}_{t}") if t == 0 else None
                      for t in range(QT_TILES)] for j in range(2)]
            KT_sb = [[apool.tile([128, 512], dt.bfloat16, name=f"KT{j}_{t}",
                                 tag=f"KT{j}_{t}") if t == 0 else None
                      for t in range(QT_TILES)] for j in range(2)]
            QT8_sb = [[apool.tile([128, 2, 512], dt.float8e4,
                                  name=f"QT8{j}_{t}", tag=f"QT8{j}_{t}")
                       if t > 0 else None for t in range(QT_TILES)]
                      for j in range(2)]
            KT8_sb = [[apool.tile([128, 2, 512], dt.float8e4,
                                  name=f"KT8{j}_{t}", tag=f"KT8{j}_{t}")
                       for t in range(QT_TILES)] for j in range(2)]
            # dim1=1 zero-fills are emitted lazily at each group's start
            # (Pool would otherwise serialize 14 memsets ahead of the first
            # causal-mask affine_selects); tracked here to emit once.
            _ms_done = set()

            def memset_pad(tile8, key):
                if key not in _ms_done:
                    _ms_done.add(key)
                    nc.gpsimd.memset(tile8[:, 1, :], 0.0)
            # bf16 V only for key chunks 0-3 (tile-0's PV stays bf16: its
            # low-query rows average over too few keys to absorb fp8 noise)
            V_sb = [apool.tile([128, H_PER_CORE * VW], dt.bfloat16,
                               name=f"V{k}", tag=f"V{k}") for k in range(4)]
            # fp8 V in chunk-PAIR layout [128, 2(parity), 4*VW] for the
            # DoubleRow PV matmuls of tiles t>=1 (contract 256 keys per
            # instruction at 0.5 cycles/row: 4x the bf16 pair cost)
            V2_sb = [apool.tile([128, 2, H_PER_CORE * VWP], dt.float8e4,
                                name=f"V2{k}", tag=f"V2{k}")
                     for k in range(KCH // 2)]
            OT_sb = [[apool.tile([128, 512], dt.bfloat16, name=f"OT{j}_{t}",
                                 tag=f"OT{j}_{t}") for t in range(QT_TILES)]
                     for j in range(2)]

            qk_ps = {}

            def emit_qk_proj(w, j, t, c0=0, c1=CCH):
                # t=0: bf16, supports partial c-ranges with a persistent
                # psum group.  t>=1: fp8 DoubleRow -- 4 matmuls each
                # contracting a 256-row c-chunk-pair at 0.5 cycles/row.
                b_sb = (bqs_sb, bks_sb)[w]
                key = (w, j, t)
                if key not in qk_ps:
                    qk_ps[key] = mmpool.tile([128, 512], dt.float32,
                                             name="ps", tag="ps")
                ps = qk_ps[key]
                woff = w * DH + j * 128
                woff8 = 128 * w
                if t == 0:
                    for c in range(c0, c1):
                        nc.tensor.matmul(
                            ps[:],
                            Wqk_sb[:, c, woff:woff + 128],
                            xT_t[t][:, c, :],
                            start=(c == 0), stop=(c == CCH - 1),
                            skip_group_check=True)
                    if c1 < CCH:
                        return
                else:
                    # c0/c1 are cp-pair indices here (0..4): partial ranges
                    # let the tile-1 prefix interleave Q and K chunk-pairs
                    # so both track the split x1 DMA arrivals
                    cp0, cp1 = (c0, min(c1, CCH // 2))
                    for cp in range(cp0, cp1):
                        nc.tensor.matmul(
                            ps[:],
                            Wqk8_sb[j][:, cp, :].rearrange(
                                "p (i d) -> p i d", i=2)[:, :,
                                                         woff8:woff8 + 128],
                            xT_t[t][:, cp, :, :],
                            start=(cp == 0), stop=(cp == CCH // 2 - 1),
                            perf_mode=mybir.MatmulPerfMode.DoubleRow,
                            skip_group_check=True)
                    if cp1 < CCH // 2:
                        return
                if w == 0:
                    d = QT_sb[j][t][:] if t == 0 else QT8_sb[j][t][:, 0, :]
                else:
                    d = KT_sb[j][t][:] if t == 0 else KT8_sb[j][t][:, 0, :]
                nc.vector.tensor_scalar_add(d, ps[:], b_sb[:, j:j + 1])
                del qk_ps[key]

            def emit_k03_f8(j):
                # fp8-projected K chunks 0-3, consumed only by tiles >= 1
                # (their rows average over >=512 keys, so the fp8 projection
                # noise is safe; tile 0 keeps its bf16 K)
                ps = mmpool.tile([128, 512], dt.float32, name="ps", tag="ps")
                for cp in range(CCH // 2):
                    nc.tensor.matmul(
                        ps[:],
                        Wqk8_sb[j][:, cp, :].rearrange(
                            "p (i d) -> p i d", i=2)[:, :, 128:256],
                        x08_sb[:, cp, :, :],
                        start=(cp == 0), stop=(cp == CCH // 2 - 1),
                        perf_mode=mybir.MatmulPerfMode.DoubleRow,
                        skip_group_check=True)
                nc.vector.tensor_scalar_add(KT8_sb[j][0][:, 0, :], ps[:],
                                            bks_sb[:, j:j + 1])

            def emit_v_proj(k):
                ps = mmpool.tile([128, DH], dt.float32, name="ps", tag="ps")
                if k < 4:
                    for c in range(CCH):
                        nc.tensor.matmul(
                            ps[:],
                            xT_t[0][:, c, (k % 4) * 128:(k % 4 + 1) * 128],
                            Wv_sb[:, c, :],
                            start=(c == 0), stop=(c == CCH - 1))
                else:
                    for cp in range(CCH // 2):
                        nc.tensor.matmul(
                            ps[:],
                            xT_t[k // 4][:, cp, :,
                                         (k % 4) * 128:(k % 4 + 1) * 128],
                            Wv8_sb[:, cp, :].rearrange(
                                "p (i e) -> p i e", i=2),
                            start=(cp == 0), stop=(cp == CCH // 2 - 1),
                            perf_mode=mybir.MatmulPerfMode.DoubleRow,
                            skip_group_check=True)
                dsts = [(V2_sb[k // 2][:, k % 2, :], VWP)]
                if k < 4:
                    dsts.append((V_sb[k][:], VW))
                for dst, vw in dsts:
                    v_dst = dst.rearrange("p (h e) -> p h e", e=vw)[:, :, 0:D_K]
                    nc.vector.tensor_tensor(
                        out=v_dst,
                        in0=ps[:].rearrange("p (h e) -> p h e", e=D_K),
                        in1=bvb_sb.rearrange("p (h e) -> p h e", e=D_K),
                        op=mybir.AluOpType.add)
                    od = dst.rearrange("p (h e) -> p h e", e=vw)[:, :, D_K]
                    nc.vector.tensor_scalar(
                        out=od, in0=bqkv_sb[:, 4:4 + H_PER_CORE], scalar1=0.0,
                        scalar2=1.0, op0=mybir.AluOpType.mult,
                        op1=mybir.AluOpType.add)

            # work queues drained into the exp-paced attention stages under
            # a per-stage PE-cost budget: projection groups for future tiles
            # first (they gate those tiles' attention), then output-projection
            # halves.  Tags order the force points: tile t start forces <= t,
            # the first PV flush of group (t, j0) forces <= t+0.5 (V tiles),
            # group (t, j1) start forces <= t+0.75 (its Q/K).
            pending_proj = []   # [tag, pe_cost_ns, fn, key]
            pending_oproj = []

            def force_item(key):
                # run (and remove) every queued item producing `key`, in
                # queue order (split projection groups share a key)
                for it in [x for x in pending_proj if x[3] == key]:
                    pending_proj.remove(it)
                    it[2]()

            def drain_budget(budget, sidx=99, allow_oproj=True,
                             on_act=False, reserve_oproj=0):
                spent = 0
                if budget <= 0:
                    return 0
                while True:
                    # first queued item whose DMA inputs have landed by
                    # this stage (draining earlier would head-of-line
                    # stall the in-order PE on the DMA semaphore)
                    it = next((x for x in pending_proj if x[4] <= sidx),
                              None)
                    if it is None or (spent and spent + it[1] > budget):
                        break
                    pending_proj.remove(it)
                    it[2]()
                    spent += it[1]
                while allow_oproj and len(pending_oproj) > reserve_oproj and spent + OPROJ_COST <= budget:
                    emit_oproj_half(*pending_oproj.pop(0), on_act=on_act)
                    spent += OPROJ_COST
                return spent

            def drain_all(on_act=False):
                while pending_proj:
                    pending_proj.pop(0)[2]()
                if TAIL_PAIR:
                    # merge TWO consecutive output rows into ONE strided
                    # dma (HWDGE costs 625ns per dma_start and the tail's
                    # issue chain ends the kernel); copies split ACT/DVE;
                    # rows alternate between the mm psum bufs and the pv
                    # banks (free after the last normalize)
                    row = 0
                    # reserved leftovers from earlier tiles can sit at the
                    # front half-aligned and would break every merge below:
                    # emit them as singles first
                    while pending_oproj and (
                            pending_oproj[0][2] == 1 or
                            len(pending_oproj) < 2 or
                            pending_oproj[1][1] != pending_oproj[0][1]):
                        emit_oproj_half(*pending_oproj.pop(0),
                                        on_act=(row % 2 == 0))
                        row += 1
                    while len(pending_oproj) >= 4 and \
                            pending_oproj[0][0] == pending_oproj[3][0] and \
                            pending_oproj[0][1] + 1 == pending_oproj[2][1]:
                        (t_, tt, _) = pending_oproj.pop(0)
                        for _ in range(3):
                            pending_oproj.pop(0)
                        o_sb = opool.tile([128, 2, 2, 512], dt.bfloat16,
                                          name="osb4", tag="osb4", bufs=3)
                        for r in range(2):
                            for m in range(2):
                                row += 1
                                if row % 2:
                                    ps = mmpool.tile([128, 512], dt.float32,
                                                     name="ps", tag="ps")
                                else:
                                    ps = pvpool.tile([128, 512], dt.float32,
                                                     name=f"pv{m}",
                                                     tag=f"pv{m}",
                                                     bufs=PV_BUFS)
                                for jj in range(2):
                                    nc.tensor.matmul(
                                        ps[:],
                                        OT_sb[jj][t_][:, ((tt + r) % 4) * 128:
                                                      ((tt + r) % 4 + 1) * 128],
                                        WoT_sb[jj][:, m * 512:(m + 1) * 512],
                                        start=(jj == 0), stop=(jj == 1))
                                if (r + m) % 2 == 0:
                                    nc.scalar.copy(o_sb[:, r, m, :], ps[:])
                                else:
                                    nc.vector.tensor_copy(o_sb[:, r, m, :],
                                                          ps[:])
                        nc.sync.dma_start(
                            out[tt * 128:(tt + 2) * 128, :].rearrange(
                                "(r p) m -> p r m", p=128),
                            o_sb[:])
                    while len(pending_oproj) >= 2 and \
                            pending_oproj[0][1] == pending_oproj[1][1]:
                        (t_, tt, _) = pending_oproj.pop(0)
                        pending_oproj.pop(0)
                        row += 1
                        o_sb = opool.tile([128, 2, 512], dt.bfloat16,
                                          name="osb2", tag="osb2", bufs=4)
                        for m in range(2):
                            if row % 2:
                                ps = mmpool.tile([128, 512], dt.float32,
                                                 name="ps", tag="ps")
                            else:
                                ps = pvpool.tile([128, 512], dt.float32,
                                                 name=f"pv{m}",
                                                 tag=f"pv{m}", bufs=PV_BUFS)
                            for jj in range(2):
                                nc.tensor.matmul(
                                    ps[:],
                                    OT_sb[jj][t_][:, (tt % 4) * 128:
                                                  (tt % 4 + 1) * 128],
                                    WoT_sb[jj][:, m * 512:(m + 1) * 512],
                                    start=(jj == 0), stop=(jj == 1))
                            if m == 0:
                                nc.scalar.copy(o_sb[:, m, :], ps[:])
                            else:
                                nc.vector.tensor_copy(o_sb[:, m, :], ps[:])
                        nc.sync.dma_start(out[tt * 128:(tt + 1) * 128, :],
                                          o_sb[:])
                i = 0
                while pending_oproj:
                    emit_oproj_half(*pending_oproj.pop(0),
                                    on_act=(i % 2 == 0),
                                    use_sc=USC(i))
                    i += 1

            def emit_oproj_half(t, tt, m, on_act=False, use_sc=False):
                # one m-half (512 of 1024 output dims) of a 128-row slice of
                # the output projection: 2 matmuls + psum->sbuf copy + DMA.
                # Half-granularity gives the t3 drain pacing enough
                # resolution to keep every exp-paced stage PE-bound.
                o_sb = opool.tile([128, 512], dt.bfloat16, name="osb",
                                  tag="osb", bufs=OSB_BUFS)
                if use_sc:
                    # endgame: the sc psum pool is idle after the final exp;
                    # borrowing its banks doubles the psum tiles in flight so
                    # the tail matmuls stop waiting on copy-recycled mm bufs
                    ps = scpool.tile([128, 2, 512], dt.float32, name="sc",
                                     tag="sc")[:, 0, :]
                else:
                    ps = mmpool.tile([128, 512], dt.float32, name="ps",
                                     tag="ps")
                for j in range(2):
                    nc.tensor.matmul(
                        ps[:],
                        OT_sb[j][t][:, (tt % 4) * 128:(tt % 4 + 1) * 128],
                        WoT_sb[j][:, m * 512:(m + 1) * 512],
                        start=(j == 0), stop=(j == 1))
                # tail halves copy on ACT (idle after the last exp; DVE is
                # busy with the final normalize chain); halves drained
                # during attention stages copy on DVE (ACT is exp-bound)
                if on_act == "split":
                    # very last half: quarter-split the copy across ACT+DVE
                    # and DMA each quarter out as it lands, shortening the
                    # end-of-kernel copy->DMA->sem chain
                    nc.scalar.copy(o_sb[:, 0:256], ps[:, 0:256])
                    nc.sync.dma_start(
                        out[tt * 128:(tt + 1) * 128,
                            m * 512:m * 512 + 256], o_sb[:, 0:256])
                    nc.vector.tensor_copy(o_sb[:, 256:512], ps[:, 256:512])
                    nc.sync.dma_start(
                        out[tt * 128:(tt + 1) * 128,
                            m * 512 + 256:(m + 1) * 512], o_sb[:, 256:512])
                    return
                if on_act:
                    nc.scalar.copy(o_sb[:], ps[:])
                else:
                    nc.vector.tensor_copy(o_sb[:], ps[:])
                nc.sync.dma_start(
                    out[tt * 128:(tt + 1) * 128, m * 512:(m + 1) * 512],
                    o_sb[:])

            from functools import partial

            def emit_filler():
                for _ in range(N_FILLER):
                    fps = scpool.tile([128, 2, 512], dt.float32, name="sc",
                                      tag="sc")
                    nc.tensor.matmul(fps[:, 0, :], warm_sb[:, 0:128],
                                     warm_sb[:], start=True, stop=True,
                                     skip_group_check=True)

            # global projection work queue, ordered by first use under the
            # [1, 2, 0, 3] tile processing order.  Keys let the attention
            # loop force exactly the group a pair is about to consume.
            TO = [int(c) for c in os.environ.get("K_TO", "1203")]

            def q(tag, cost, fn, key, ready=0):
                pending_proj.append([tag, cost, fn, key, ready])

            # tile-1 j0 prefix, emitted directly in cp-interleaved order:
            # Q/K chunk-pairs 0-1 run on the first x1 half-DMA while
            # chunk-pairs 2-3 wait for the second
            emit_qk_proj(0, 0, 1, 0, 2)
            emit_qk_proj(1, 0, 1, 0, 2)
            emit_qk_proj(0, 0, 1, 2, 4)
            emit_qk_proj(1, 0, 1, 2, 4)
            q(1, 427, partial(emit_k03_f8, 0), ("K8", 0, 0), 1)
            q(1, 427, partial(emit_k03_f8, 1), ("K8", 1, 0), 1)
            q(1, 427, partial(emit_qk_proj, 0, 1, 1), ("Q", 1, 1), 1)
            q(1, 427, partial(emit_qk_proj, 1, 1, 1), ("K", 1, 1), 1)
            for k in (4, 5, 6, 7):
                q(1, 250, partial(emit_v_proj, k), ("V", k), 2)
            for k in (0, 1, 2, 3):
                q(1, 900, partial(emit_v_proj, k), ("V", k), 4)
            q(2, 427, partial(emit_qk_proj, 0, 0, 2), ("Q", 0, 2), 2)
            q(2, 427, partial(emit_qk_proj, 1, 0, 2), ("K", 0, 2), 2)
            for k in (8, 9, 10, 11):
                q(2, 250, partial(emit_v_proj, k), ("V", k), 2)
            q(2, 427, partial(emit_qk_proj, 0, 1, 2), ("Q", 1, 2), 2)
            q(2, 427, partial(emit_qk_proj, 1, 1, 2), ("K", 1, 2), 2)
            q(3, 853, partial(emit_qk_proj, 1, 0, 0, 0, 4), ("K", 0, 0), 5)
            q(3, 900, partial(emit_qk_proj, 1, 0, 0, 4, 8), ("K", 0, 0), 5)
            q(3, 853, partial(emit_qk_proj, 0, 0, 0, 0, 4), ("Q", 0, 0), 6)
            q(3, 900, partial(emit_qk_proj, 0, 0, 0, 4, 8), ("Q", 0, 0), 6)
            q(3, 853, partial(emit_qk_proj, 1, 1, 0, 0, 4), ("K", 1, 0), 5)
            q(3, 900, partial(emit_qk_proj, 1, 1, 0, 4, 8), ("K", 1, 0), 5)
            q(3, 853, partial(emit_qk_proj, 0, 1, 0, 0, 4), ("Q", 1, 0), 6)
            q(3, 900, partial(emit_qk_proj, 0, 1, 0, 4, 8), ("Q", 1, 0), 6)
            q(4, 427, partial(emit_qk_proj, 0, 0, 3), ("Q", 0, 3), 3)
            q(4, 427, partial(emit_qk_proj, 1, 0, 3), ("K", 0, 3), 3)
            for k in (12, 13, 14, 15):
                q(4, 250, partial(emit_v_proj, k), ("V", k), 3)
            q(4, 427, partial(emit_qk_proj, 0, 1, 3), ("Q", 1, 3), 3)
            q(4, 427, partial(emit_qk_proj, 1, 1, 3), ("K", 1, 3), 3)

            for j in range(2):
                nc.sync.dma_start(WoT_sb[j][:],
                                  WoT[j * 128:(j + 1) * 128, :])
            def flush(mms_pp, final):
                for p in range(2):
                    for i, (lhsT, rhs, o, st, pm) in enumerate(mms_pp[p]):
                        nc.tensor.matmul(
                            o, lhsT, rhs, start=st,
                            stop=(final and i == len(mms_pp[p]) - 1),
                            perf_mode=pm, skip_group_check=True)

            pending_norm = []

            def normalize_group(t, j, pvs):
                # OT = PV * (1/denom).  DVE tensor_tensor can read only ONE
                # psum operand, so the broadcast goes through gpsimd to
                # SBUF.  The mults are DEFERRED one stage: an in-order DVE
                # would otherwise head-of-line stall on the Pool broadcast
                # roundtrip, delaying the next group's Q/K bias write.
                bcs = []
                for p in range(2):
                    rc = smpool.tile([1, 512], dt.float32, name="rc",
                                     tag="rc", bufs=SM_BUFS)
                    bc = smpool.tile([64, 512], dt.float32, name="bc",
                                     tag="bc", bufs=SM_BUFS)
                    nc.vector.reciprocal(rc[:], pvs[p][D_K:VW, :])
                    nc.gpsimd.partition_broadcast(bc[:], rc[:])
                    bcs.append(bc)

                def mults():
                    for p in range(2):
                        nc.vector.tensor_tensor(
                            out=OT_sb[j][t][p * 64:(p + 1) * 64, :],
                            in0=pvs[p][0:D_K, :], in1=bcs[p][:],
                            op=mybir.AluOpType.mult)
                pending_norm.append(mults)

            def normalize_last(t, j, pvs):
                # final group: the first 128 columns get their own short
                # recip/bcast/mult so the first tail oproj half (which
                # reads OT cols 0:128) unblocks as early as possible
                rcs = [smpool.tile([1, 512], dt.float32, name="rc",
                                   tag="rc", bufs=SM_BUFS) for _ in range(2)]
                bcs = [smpool.tile([64, 512], dt.float32, name="bc",
                                   tag="bc", bufs=SM_BUFS) for _ in range(2)]
                for hh in range(2):
                    s = slice(hh * 256, (hh + 1) * 256)
                    for p in range(2):
                        nc.vector.reciprocal(rcs[p][:, s], pvs[p][D_K:VW, s])
                        nc.gpsimd.partition_broadcast(bcs[p][:, s],
                                                      rcs[p][:, s])
                    for cc in (2 * hh, 2 * hh + 1):
                        sc_ = slice(cc * 128, (cc + 1) * 128)
                        for p in range(2):
                            nc.vector.tensor_tensor(
                                out=OT_sb[j][t][p * 64:(p + 1) * 64, sc_],
                                in0=pvs[p][0:D_K, sc_], in1=bcs[p][:, sc_],
                                op=mybir.AluOpType.mult)

            # ---- global stage pipeline: one stage per (tile, head-pair,
            # key-chunk-pair).  The PV-flush software pipeline runs ACROSS
            # group and tile boundaries, so a group's flush+normalize tail
            # overlaps the next group's scores/exp instead of serializing
            # at each boundary.  Diagonal pairs first within each group.
            stages = []
            for t in TO:
                for j in range(2):
                    pis = [2 * t, 2 * t + 1] + list(range(2 * t))
                    for pii, pi in enumerate(pis):
                        stages.append((t, j, pi, pii, len(pis)))

            pend = []          # (t, j, pi, mms_pp, pvs, final)
            group_pvs = {}
            LAST = (TO[-1], 1)

            def pop_flush():
                t_, j_, pi_, mms_, pvs_, final_ = pend.pop(0)
                force_item(("V", 2 * pi_))
                force_item(("V", 2 * pi_ + 1))
                flush(mms_, final_)
                if final_:
                    if (t_, j_) == LAST:
                        normalize_last(t_, j_, pvs_)
                    else:
                        normalize_group(t_, j_, pvs_)
                    if j_ == 1:
                        # both head-pairs' OT ready: queue the tile's
                        # output-projection halves
                        for tt in range(4 * t_, 4 * t_ + 4):
                            for m in range(2):
                                pending_oproj.append((t_, tt, m))

            for sidx, (t, j, pi, pii, npair) in enumerate(stages):
                if pii == 0:
                    if (t, j) == (TO[0], 0):
                        # head-critical zero-fills: tile 1's own k-tile
                        # padding plus the chunk-0-3 K it reads
                        for jj in range(2):
                            memset_pad(QT8_sb[jj][1], ("q", jj, 1))
                            memset_pad(KT8_sb[jj][1], ("k", jj, 1))
                            memset_pad(KT8_sb[jj][0], ("k", jj, 0))
                    elif j == 1 and t in (TO[0], TO[1]):
                        # prefetch the NEXT tile's pads a whole group early
                        # so Pool has slack to run them behind the masks
                        tn = TO[TO.index(t) + 1]
                        for jj in range(2):
                            if tn > 0:
                                memset_pad(QT8_sb[jj][tn], ("q", jj, tn))
                            memset_pad(KT8_sb[jj][tn], ("k", jj, tn))
                    elif j == 0 and t == TO[2]:
                        for jj in range(2):
                            memset_pad(QT8_sb[jj][3], ("q", jj, 3))
                            memset_pad(KT8_sb[jj][3], ("k", jj, 3))
                    # this group's own Q/K must exist before its scores
                    force_item(("Q", j, t))
                    force_item(("K", j, t))
                    group_pvs[(t, j)] = [
                        pvpool.tile([128, 512], dt.float32, name=f"pv{p}",
                                    tag=f"pv{p}", bufs=PV_BUFS)
                        for p in range(2)]
                pvs = group_pvs[(t, j)]
                diag = pi >= 2 * t
                first_pair = pii == 0
                # for the second diagonal pair (r0=2) only columns
                # q >= 128*r0 can be unmasked for either half, so the
                # scores matmuls and exp skip the dead columns.
                q0 = 128 * 2 * (pi - 2 * t) if diag else 0
                use8 = t > 0  # fp8 DoubleRow scores+PV for tiles >= 1
                pt_dt = dt.float8e4 if use8 else dt.bfloat16
                pt_pool = pt8pool if use8 else ptpool
                # the K projection tile this pair's scores read
                if use8 and pi // 2 == 0:
                    force_item(("K8", j, 0))
                else:
                    force_item(("K", j, pi // 2))
                pts = []
                for p in range(2):
                    lo = p * 64
                    sc = scpool.tile([128, 2, 512], dt.float32,
                                     name="sc", tag="sc")
                    for half in range(2):
                        c = 2 * pi + half
                        # per-half trim: k-block r = 2(pi-2t)+half is only
                        # unmasked for q >= 128*r
                        q0h = 128 * (2 * (pi - 2 * t) + half) \
                            if diag else 0
                        if use8:
                            nc.tensor.matmul(
                                sc[:, half, q0h:],
                                KT8_sb[j][(2 * pi) // 4][
                                    lo:lo + 64, :,
                                    (c % 4) * 128:(c % 4 + 1) * 128],
                                QT8_sb[j][t][lo:lo + 64, :, q0h:],
                                start=True, stop=True,
                                tile_position=(lo, 0),
                                perf_mode=mybir.MatmulPerfMode.DoubleRow)
                        else:
                            nc.tensor.matmul(
                                sc[:, half, q0h:],
                                KT_sb[j][(2 * pi) // 4][
                                    lo:lo + 64,
                                    (c % 4) * 128:(c % 4 + 1) * 128],
                                QT_sb[j][t][lo:lo + 64, q0h:],
                                start=True, stop=True,
                                tile_position=(lo, 0))
                    pt = pt_pool.tile([128, 2, 512], pt_dt,
                                      name="pt", tag="pt")
                    nc.scalar.activation(pt[:, :, q0:],
                                         sc[:, :, q0:], AF.Exp)
                    pts.append(pt)
                mms_pp = []
                for p in range(2):
                    pt = pts[p]
                    h = 2 * j + p
                    vsl = lambda c: V_sb[c][:, h * VW:(h + 1) * VW]
                    mms = []  # (lhsT, rhs, out, start, perf_mode)
                    if use8:
                        v2 = V2_sb[pi][:, :, h * VWP:h * VWP + VW]
                        if diag:
                            r0 = 2 * (pi - 2 * t)
                            # half0: zero below-diagonal of its 128x128
                            # triangle block
                            tri = pt[:, 0, 128 * r0:128 * (r0 + 1)]
                            nc.gpsimd.affine_select(
                                out=tri, in_=tri,
                                compare_op=mybir.AluOpType.is_ge,
                                fill=0.0, base=0, pattern=[[1, 128]],
                                channel_multiplier=-1)
                            # half1: cols [128r0, 128(r0+1)) are entirely
                            # below-diagonal plus its own triangle block --
                            # one widened select covers both
                            tri2 = pt[:, 1, 128 * r0:128 * (r0 + 2)]
                            nc.gpsimd.affine_select(
                                out=tri2, in_=tri2,
                                compare_op=mybir.AluOpType.is_ge,
                                fill=0.0, base=-128, pattern=[[1, 256]],
                                channel_multiplier=-1)
                            mms.append((v2, pt[:, :, 128 * r0:],
                                        pvs[p][0:VW, 128 * r0:],
                                        first_pair,
                                        mybir.MatmulPerfMode.DoubleRow))
                        else:
                            mms.append((v2, pt[:, :, :],
                                        pvs[p][0:VW, :], first_pair,
                                        mybir.MatmulPerfMode.DoubleRow))
                    elif not diag:
                        for half in range(2):
                            mms.append((vsl(2 * pi + half),
                                        pt[:, half, :], pvs[p][0:VW, :],
                                        first_pair and half == 0, None))
                    else:
                        r0 = 2 * (pi - 2 * t)
                        for half in range(2):
                            r = r0 + half
                            tri = pt[:, half, 128 * r:128 * (r + 1)]
                            nc.gpsimd.affine_select(
                                out=tri, in_=tri,
                                compare_op=mybir.AluOpType.is_ge,
                                fill=0.0, base=0, pattern=[[1, 128]],
                                channel_multiplier=-1)
                        for half in range(2):
                            r = r0 + half
                            mms.append((vsl(2 * pi + half),
                                        pt[:, half, 128 * r:],
                                        pvs[p][0:VW, 128 * r:],
                                        first_pair and half == 0, None))
                    mms_pp.append(mms)
                pend.append((t, j, pi, mms_pp, pvs, pii == npair - 1))
                while pending_norm:
                    pending_norm.pop(0)()
                drain_budget(BUD_T0 if t == 0 else BUD, sidx=sidx,
                             reserve_oproj=RESERVE)
                # the LAST group flushes shallow: with the full depth its
                # pairs would all queue behind the final exp, serializing
                # flush+normalize+output-projection into the tail
                depth = PV_LAST if (t, j) == LAST else PV_DEPTH
                while len(pend) > depth:
                    pop_flush()
            # pipeline tail: flush the last pairs, then the reserved oproj
            # halves fill the final normalize window
            while pend:
                pop_flush()
                while pending_norm:
                    pending_norm.pop(0)()
            drain_all()
            if TAIL_PAIR:
                # tail: one o_sb + one DMA per tt row-block (the 8 tail
                # DMAs otherwise serialize on the 625ns-per-DMA HWDGE)
                for i in range(0, len(pending_oproj), 2):
                    (t_, tt, _), _ = pending_oproj[i], pending_oproj[i + 1]
                    o_sb = opool.tile([128, 2, 512], dt.bfloat16,
                                      name="osb2", tag="osb2", bufs=2)
                    for m in range(2):
                        ps = mmpool.tile([128, 512], dt.float32, name="ps",
                                         tag="ps")
                        for j in range(2):
                            nc.tensor.matmul(
                                ps[:],
                                OT_sb[j][t_][:,
                                             (tt % 4) * 128:(tt % 4 + 1) * 128],
                                WoT_sb[j][:, m * 512:(m + 1) * 512],
                                start=(j == 0), stop=(j == 1))
                        if m == 0:
                            nc.scalar.copy(o_sb[:, m, :], ps[:])
                        else:
                            nc.vector.tensor_copy(o_sb[:, m, :], ps[:])
                    nc.sync.dma_start(out[tt * 128:(tt + 1) * 128, :],
                                      o_sb[:])
            else:
                for i, args in enumerate(pending_oproj):
                    last = i == len(pending_oproj) - 1
                    emit_oproj_half(*args,
                                    on_act="split" if (last and SPLIT_LAST)
                                    else OACT(i),
                                    use_sc=USC(i))
    nc.compile()
    return nc


def _in_maps(x, Wq, bq, Wk, bk, Wv, bv, Wo, bo):
    maps = []
    F8 = ml_dtypes.float8_e4m3fn
    dts = {"xT0": BF16, "xT8": F8, "x08T": F8, "WqkT": BF16,
           "Wqk8AT": F8, "Wqk8BT": F8,
           "WvT": BF16, "Wv8T": F8, "WoT": BF16, "bqkv": np.float32}
    for core in range(N_CORES):
        b = core // 4
        h0 = (core % 4) * H_PER_CORE
        hs = slice(h0 * D_K, (h0 + H_PER_CORE) * D_K)
        xTb = np.ascontiguousarray(x[b].T)
        wqk = np.concatenate([
            np.ascontiguousarray(Wq[hs, :].T) * 0.125,
            np.ascontiguousarray(Wk[hs, :].T)], axis=1)
        wv = np.ascontiguousarray(Wv[hs, :].T)
        m = {
            "xT0": xTb[:, 0:512],
            "xT8": xTb[:, 512:],
            "x08T": xTb[:, 0:512],
            "WqkT": wqk,
            "Wqk8AT": np.concatenate([wqk[:, 0:128], wqk[:, 256:384]],
                                     axis=1).reshape(
                CCH // 2, 2, 128, 256).transpose(0, 2, 1, 3).reshape(
                D_MODEL // 2, 512),
            "Wqk8BT": np.concatenate([wqk[:, 128:256], wqk[:, 384:512]],
                                     axis=1).reshape(
                CCH // 2, 2, 128, 256).transpose(0, 2, 1, 3).reshape(
                D_MODEL // 2, 512),
            "WvT": wv,
            "Wv8T": wv.reshape(CCH // 2, 2, 128, DH).transpose(
                0, 2, 1, 3).reshape(D_MODEL // 2, 2 * DH),
            "WoT": np.ascontiguousarray(Wo[:, hs].T),
            "bqkv": np.concatenate([
                np.ascontiguousarray((bq[hs] * 0.125).reshape(2, 128).T),
                np.ascontiguousarray(bk[hs].reshape(2, 128).T),
                np.broadcast_to(bv[hs], (128, DH)),
            ], axis=1),
        }
        maps.append({k: np.ascontiguousarray(v, dtype=dts[k])
                     for k, v in m.items()})
    return maps


def kernel(x, Wq, bq, Wk, bk, Wv, bv, Wo, bo, _trace=False):
    if "nc" not in _CACHE:
        _CACHE["nc"] = _build()
    nc = _CACHE["nc"]
    in_maps = _in_maps(np.asarray(x, dtype=np.float32),
                       np.asarray(Wq, dtype=np.float32),
                       np.asarray(bq, dtype=np.float32),
                       np.asarray(Wk, dtype=np.float32),
                       np.asarray(bk, dtype=np.float32),
                       np.asarray(Wv, dtype=np.float32),
                       np.asarray(bv, dtype=np.float32),
                       np.asarray(Wo, dtype=np.float32),
                       np.asarray(bo, dtype=np.float32))
    res = run_bass_kernel_spmd(nc, in_maps, core_ids=list(range(N_CORES)),
                               trace=_trace)
    bo = np.asarray(bo, dtype=np.float32)
    out = np.zeros((B, S, D_MODEL), dtype=np.float32)
    for b in range(B):
        acc = res.results[b * 4]["out"].astype(np.float64)
        for core in range(b * 4 + 1, b * 4 + 4):
            acc = acc + res.results[core]["out"]
        out[b] = (acc + bo).astype(np.float32)
    if _trace:
        return out, res
    return out



# revision 64
# speedup vs baseline: 1.0266x; 1.0008x over previous
"""MultiHeadAttention (B=2, S=2048, d_model=1024, 16 heads, causal) on 8 TRN2 cores.

Sharding: core i handles batch (i//4) and heads 4*(i%4) .. 4*(i%4)+4 (tensor
parallel over heads within a batch).  Each core computes its 4 heads'
Q/K/V projections, causal attention, and the partial output projection
(contribution of its 256 head-dims to all 1024 output dims).  The host sums
the 4 bf16 partials per batch in float64 and adds the output bias.

Precision strategy (validated against the fp32 reference in numpy; the
harness gate is scale-relative absmax 2e-2, this kernel lands ~9.1e-3):
  - everything touching QUERIES < 512 (q-tile 0) runs bf16: those rows
    average over too few keys to absorb fp8 quantization noise (q=0 outputs
    V[0] exactly, so fp8's ~6% element error would land raw in the output).
  - tiles 1-3 run fp8e4m3 end to end: x tiles 1-3 (plus an fp8 copy of
    x tile 0 that feeds the fp8 K chunks 0-3 those tiles read, so their
    early pairs never wait on the 1MB bf16 x0 transfer) and the
    Wq/Wk/Wv copies ship fp8 from the host; Q/K/V projections, QK^T scores, and the PV
    matmul all use fp8 DoubleRow perf mode, which the PE charges at 0.5
    cycles/row while contracting TWO 128-row k-tiles per instruction:
      * projections: c-chunk pairs packed -> 4x cheaper than bf16
      * PV: two key-chunks packed ([128, 2(parity), 4*68] fp8 V layout,
        head stride padded to 68 for the 16B outer-stride ISA rule) -> 4x
      * scores: d_k=64 only fills half the array, so the second k-tile is
        ZERO-padded (KT8/QT8 are [128, 2, 512] with dim1=1 memset once;
        0 * garbage would be NaN-poisoned otherwise) -> 2x
    Errors average out over >=512 keys: the fp8 kernel's absmax error is
    within 2.5x of the all-bf16 kernel's.
  - the output projection stays bf16 (fp8 fails the gate: contraction 256
    gives no averaging), PSUM accumulation is fp32 throughout, exp output
    is written directly as fp8 for tiles >= 1 (free on ACT).
  - bias handling: 1/sqrt(d_k) folds into Wq/bq on the host; V's bias rides
    a broadcast row of the bqkv tensor; the softmax denominator comes from
    a ones-column appended to V (accumulated by the same PV matmuls).

The machine balance after fp8: ACT (exp) ~77.5us busy is the bottleneck
(PE ~59, DVE ~60, Pool ~43), so the whole kernel is scheduled as ONE
global pipeline of 40 exp-paced stages (one per (tile, head-pair,
key-chunk-pair)):
  - tiles process in order [1, 2, 0, 3]: the j0 half of the fp8 Wqk
    plus fp8 x1 is all the first exp needs, so it lands first and the
    exp stream starts at ~7.4us (vs ~15us for the original bf16-first
    supply).  The DMA chain is sequenced by strict NEED time -- all fp8
    supply (x08, Wqk8B, Wv8, x82, x83) ships before the bf16 tile-0
    supply (Wv, x0, Wqk columns), which nothing reads before stage ~20.
    The whole exp stream is anchored to DMA landing times, so every ns
    squeezed out of the chain moves the stream end 1:1.  fp8 tensors are
    host-packed (two c-rows per DRAM row) so both DMA sides keep >=512B
    contiguous runs (256B runs pay a 2x descriptor-latency penalty).
  - PV flushes run a 7-deep software pipeline ACROSS group and tile
    boundaries: a group's flush + normalize tail overlaps the next
    group's scores/exp instead of serializing at each boundary.  A
    group's normalize MULTS are deferred one stage so the in-order DVE
    never head-of-line stalls on the Pool broadcast roundtrip.
  - projection/output-projection work drains into each stage under a PE
    budget (~1000ns) with per-item DMA-readiness stages (draining a
    not-yet-landed item would head-of-line stall the in-order PE), and
    keyed force points guarantee a group's Q/K/V exist exactly when its
    scores or flushes consume them.
  - causal masking: per-half column trims on the diagonal pairs plus
    gpsimd affine_selects on the two 128x128 triangle blocks (the fp8
    path widens one select to also zero the below-diagonal block that
    column ranges used to exclude).
  - the endgame pairs output rows into single DMAs (HWDGE costs 625ns
    per dma_start) EXCEPT the final row, which goes as two half DMAs so
    each issues the moment its psum->sbuf copy lands; copies split
    ACT/DVE, psum alternates between the mm and then-idle pv pools, and
    the last normalize is column-split so the tail starts early.  No
    oproj reserve: the tail queue must stay row-aligned or the pair
    merges all fall through to issue-chain-gated single DMAs.

Cost-model (TimelineSim) estimate: 98.42 us/core (fp32r baseline kernel:
146.0 us; bf16 predecessor: 130.6 us).  Scale-relative absmax error vs
the fp32 reference: 9.29e-3 (gate is 2e-2).
"""

import numpy as np
import ml_dtypes

import concourse.bass as bass
import concourse.tile as tile
import concourse.mybir as mybir
from concourse import bacc
from concourse.bass_utils import run_bass_kernel_spmd

dt = mybir.dt
AF = mybir.ActivationFunctionType
BF16 = ml_dtypes.bfloat16

D_MODEL = 1024
N_HEADS = 16
D_K = 64
B = 2
S = 2048
H_PER_CORE = 4
DH = H_PER_CORE * D_K  # 256
N_CORES = 8
CCH = D_MODEL // 128  # 8 contraction chunks
QT_TILES = S // 512  # 4
KCH = S // 128  # 16 key chunks
VW = D_K + 1  # 65
VWP = 68  # fp8 V per-head stride: DoubleRow needs 16B-aligned outer strides

_CACHE = {}

import os

CEXP = int(os.environ.get("K_CEXP", "0"))
SC_BUFS = int(os.environ.get("K_SC_BUFS", "2"))
MM_BUFS = 2
PV_BUFS = 1
PT_BUFS = int(os.environ.get("K_PT", "12"))
N_WARMUP = int(os.environ.get("K_WARMUP", "5"))
WARM_MEMSET = int(os.environ.get("K_WARM_MEMSET", "1"))
DRAIN2 = int(os.environ.get("K_DRAIN2", "0"))
FLUSH1 = int(os.environ.get("K_FLUSH1", "0"))
XPRE = int(os.environ.get("K_XPRE", "2"))
RESERVE = int(os.environ.get("K_RESERVE", "0"))
GDRAIN = int(os.environ.get("K_GDRAIN", "3"))
BUD_T0 = int(os.environ.get("K_BUD_T0", "700"))
BUD = int(os.environ.get("K_BUD", "1000"))
BUD_G = int(os.environ.get("K_BUD_G", "1300"))
OPROJ_COST = int(os.environ.get("K_OPROJ_COST", "480"))
N_FILLER = int(os.environ.get("K_FILLER", "0"))
NORM_ORDER = os.environ.get("K_NORM", "paired")
SPLIT_LAST = int(os.environ.get("K_SPLIT_LAST", "0"))
TAIL_PAIR = int(os.environ.get("K_TAIL_PAIR", "1"))
PACE_R = int(os.environ.get("K_PACE_R", "0"))
PV_DEPTH = int(os.environ.get("K_PV_DEPTH", "7"))
PV_LAST = int(os.environ.get("K_PV_LAST", "4"))
WV_SPLIT = int(os.environ.get("K_WV_SPLIT", "1"))
WV_EARLY = int(os.environ.get("K_WV_EARLY", "0"))
BQKV_EARLY = int(os.environ.get("K_BQKV_EARLY", "0"))
OSB_BUFS = int(os.environ.get("K_OSB", "6"))
SM_BUFS = int(os.environ.get("K_SM", "8"))
_oact = os.environ.get("K_OACT", "all")
OACT = {"all": lambda i: True, "mix": lambda i: i < 3 or i % 2 == 0,
        "alt": lambda i: i % 2 == 0}[_oact]
_usc = os.environ.get("K_USC", "none")
USC = {"none": lambda i: False, "alt": lambda i: i % 2 == 1,
       "all": lambda i: True}[_usc]


def _build():
    nc = bacc.Bacc("TRN2", target_bir_lowering=False, debug=False,
                   num_devices=N_CORES)

    # x tile 0 ships bf16 (tile-0 numerics need it); tiles 1-3 ship fp8.
    # Weights ship in both precisions: bf16 for the tile-0/chunk-0-3
    # projections, fp8 for the DoubleRow projections of everything else.
    xT0 = nc.dram_tensor("xT0", [D_MODEL, 512], dt.bfloat16,
                         kind="ExternalInput").ap()
    xT8 = nc.dram_tensor("xT8", [D_MODEL, S - 512], dt.float8e4,
                         kind="ExternalInput").ap()
    x08T = nc.dram_tensor("x08T", [D_MODEL, 512], dt.float8e4,
                          kind="ExternalInput").ap()
    WqkT = nc.dram_tensor("WqkT", [D_MODEL, 2 * DH], dt.bfloat16,
                          kind="ExternalInput").ap()
    Wqk8AT = nc.dram_tensor("Wqk8AT", [D_MODEL // 2, 2 * DH], dt.float8e4,
                            kind="ExternalInput").ap()
    Wqk8BT = nc.dram_tensor("Wqk8BT", [D_MODEL // 2, 2 * DH], dt.float8e4,
                            kind="ExternalInput").ap()
    WvT = nc.dram_tensor("WvT", [D_MODEL, DH], dt.bfloat16, kind="ExternalInput").ap()
    Wv8T = nc.dram_tensor("Wv8T", [D_MODEL // 2, 2 * DH], dt.float8e4,
                          kind="ExternalInput").ap()
    WoT = nc.dram_tensor("WoT", [DH, D_MODEL], dt.bfloat16, kind="ExternalInput").ap()
    bqkv = nc.dram_tensor("bqkv", [128, 4 + DH], dt.float32,
                          kind="ExternalInput").ap()
    out = nc.dram_tensor("out", [S, D_MODEL], dt.bfloat16, kind="ExternalOutput").ap()


    with tile.TileContext(nc) as tc:
        with (
            tc.tile_pool(name="weights", bufs=1) as wpool,
            tc.tile_pool(name="acts", bufs=1) as apool,
            tc.tile_pool(name="pt", bufs=PT_BUFS) as ptpool,
            tc.tile_pool(name="pt8", bufs=PT_BUFS) as pt8pool,
            tc.tile_pool(name="sm", bufs=4) as smpool,
            tc.tile_pool(name="outsb", bufs=4) as opool,
            tc.tile_pool(name="mm", bufs=MM_BUFS, space="PSUM") as mmpool,
            tc.tile_pool(name="sc", bufs=SC_BUFS, space="PSUM") as scpool,
            tc.tile_pool(name="pv", bufs=1, space="PSUM") as pvpool,
        ):
            # ---- PE warmup: the cost model's p-state ramp needs ~3us of
            # continuous PE activity before matmuls run at full clock.  The
            # first input DMAs take ~1-2us to land, so burn that window with
            # dummy matmuls on a scratch tile (never read back); real matmuls
            # queue behind them and start fully warm.
            warm_sb = wpool.tile([128, 512], dt.bfloat16, name="warm",
                                 tag="warm")
            if WARM_MEMSET:
                nc.vector.memset(warm_sb[:], 0.0)
            warm_ps = mmpool.tile([128, 512], dt.float32, name="ps", tag="ps")
            for wi in range(N_WARMUP):
                nc.tensor.matmul(warm_ps[:], warm_sb[:, 0:128], warm_sb[:],
                                 start=True, stop=True, skip_group_check=True)

            # ---- input loads, ordered by first use under the [1,2,0,3]
            # tile processing order: the fp8 Wqk + fp8 x1 land first (~3us)
            # so tile 1's DoubleRow projections and first exp start early;
            # the bf16 Wqk/x0 (tile-0 path) stream in behind them.
            # tile0 x: [128, c(8), 512] bf16; tiles 1-3: [128, cp(4),
            # 2(c-parity), 512] fp8 -- the DoubleRow k-tile pair layout.
            xT_t = [wpool.tile([128, CCH, 512], dt.bfloat16, name="xTt0",
                               tag="xTt0") if t == 0 else
                    wpool.tile([128, CCH // 2, 2, 512], dt.float8e4,
                               name=f"xTt{t}", tag=f"xTt{t}")
                    for t in range(QT_TILES)]
            xT0_r = xT0.rearrange("(c p) q -> p c q", p=128)
            x8_r = xT8.rearrange("(cp i p) q -> p cp i q", p=128, i=2)
            x08_r = x08T.rearrange("(cp i p) q -> p cp i q", p=128, i=2)
            x08_sb = wpool.tile([128, CCH // 2, 2, 512], dt.float8e4,
                                name="x08", tag="x08")
            Wqk_sb = wpool.tile([128, CCH, 2 * DH], dt.bfloat16)
            # fp8 Wqk ships as TWO host-packed half tensors (j0 = [Q|K]
    # cols of head-pair 0, j1 likewise), each [512, 512] with two
            # c-rows per DRAM row so both DMA sides keep 512B runs; the
            # j0 half plus fp8 x1 is all the first exp needs
            Wqk8_sb = [wpool.tile([128, CCH // 2, 2 * DH], dt.float8e4,
                                  name=f"W8{j}", tag=f"W8{j}")
                       for j in range(2)]
            Wv_sb = wpool.tile([128, CCH, DH], dt.bfloat16)
            # fp8 Wv ships host-packed as [512, 2*DH] (two c-rows per DRAM
            # row) so both DMA sides have 512B contiguous runs -- 256B runs
            # pay a 2x DMA latency penalty
            Wv8_sb = wpool.tile([128, CCH // 2, 2 * DH], dt.float8e4)
            wqk_r = WqkT.rearrange("(c p) d -> p c d", p=128)
            wqk8a_r = Wqk8AT.rearrange("(cp p) e -> p cp e", p=128)
            wqk8b_r = Wqk8BT.rearrange("(cp p) e -> p cp e", p=128)
            wv_r = WvT.rearrange("(c p) d -> p c d", p=128)
            wv8_r = Wv8T.rearrange("(cp p) e -> p cp e", p=128)
            bqkv_sb = wpool.tile([128, 4 + DH], dt.float32)
            # head-critical: the j0 half of the fp8 Wqk (host layout is
            # [Qj0|Kj0|Qj1|Kj1] so it is contiguous), then fp8 x1 -- these
            # gate tile 1's first scores.  bqkv (bias, needed by the first
            # DVE write) follows, then the bf16 x0 (gates the chunk-0-3
            # K/V projections tile 1's later pairs read).
            nc.sync.dma_start(Wqk8_sb[0][:], wqk8a_r)
            nc.sync.dma_start(xT_t[1][:, 0:2, :, :], x8_r[:, 0:2, :, 0:512])
            nc.sync.dma_start(xT_t[1][:, 2:4, :, :], x8_r[:, 2:4, :, 0:512])
            nc.sync.dma_start(bqkv_sb[:], bqkv)
            # fp8 copy of x tile 0: feeds the fp8 K chunks 0-3 that tiles
            # >=1 read, so their early pairs never wait on the 1MB bf16 x0
            nc.sync.dma_start(x08_sb[:], x08_r)
            nc.sync.dma_start(Wqk8_sb[1][:], wqk8b_r)
            nc.sync.dma_start(Wv8_sb[:], wv8_r)
            bqs_sb = bqkv_sb[:, 0:2]
            bks_sb = bqkv_sb[:, 2:4]
            bvb_sb = bqkv_sb[:, 4:4 + DH]
            WoT_sb = [wpool.tile([128, D_MODEL], dt.bfloat16, name=f"Wo{j}",
                                 tag=f"Wo{j}") for j in range(2)]
            # remaining fp8 x tiles next (tiles 2/3 start at stages 8/24),
            # THEN the bf16 tile-0 supply -- t0 runs third, so nothing
            # needs it before ~stage 20
            for tx in (2, 3):
                nc.sync.dma_start(
                    xT_t[tx][:],
                    x8_r[:, :, :, (tx - 1) * 512:tx * 512])
            nc.sync.dma_start(Wv_sb[:, 0:4, :], wv_r[:, 0:4, :])
            nc.sync.dma_start(Wv_sb[:, 4:8, :], wv_r[:, 4:8, :])
            nc.sync.dma_start(xT_t[0][:, 0:4, :], xT0_r[:, 0:4, :])
            nc.sync.dma_start(xT_t[0][:, 4:8, :], xT0_r[:, 4:8, :])
            nc.sync.dma_start(Wqk_sb[:, :, 256:512], wqk_r[:, :, 256:512])
            nc.sync.dma_start(Wqk_sb[:, :, 0:256], wqk_r[:, :, 0:256])

            # ---- fully streamed per q-tile: projections for tile t, then
            # attention for tile t (overlaps next tile's projections on PE),
            # then tile t's slice of the output projection.
            # bf16 Q/K only for tile 0 (its low-query rows need bf16
            # scores); fp8 zero-padded [128, 2(k-tile), 512] Q/K feed the
            # DoubleRow scores matmuls of tiles t>=1 at 0.5 cycles/row.
            # dim1=1 is memset to zero once (garbage would poison 0*NaN).
            QT_sb = [[apool.tile([128, 512], dt.bfloat16, name=f"QT{j}_{t}",
                                 tag=f"QT{j}_{t}") if t == 0 else None
                      for t in range(QT_TILES)] for j in range(2)]
            KT_sb = [[apool.tile([128, 512], dt.bfloat16, name=f"KT{j}_{t}",
                                 tag=f"KT{j}_{t}") if t == 0 else None
                      for t in range(QT_TILES)] for j in range(2)]
            QT8_sb = [[apool.tile([128, 2, 512], dt.float8e4,
                                  name=f"QT8{j}_{t}", tag=f"QT8{j}_{t}")
                       if t > 0 else None for t in range(QT_TILES)]
                      for j in range(2)]
            KT8_sb = [[apool.tile([128, 2, 512], dt.float8e4,
                                  name=f"KT8{j}_{t}", tag=f"KT8{j}_{t}")
                       for t in range(QT_TILES)] for j in range(2)]
            # dim1=1 zero-fills are emitted lazily at each group's start
            # (Pool would otherwise serialize 14 memsets ahead of the first
            # causal-mask affine_selects); tracked here to emit once.
            _ms_done = set()

            def memset_pad(tile8, key):
                if key not in _ms_done:
                    _ms_done.add(key)
                    nc.gpsimd.memset(tile8[:, 1, :], 0.0)
            # bf16 V only for key chunks 0-3 (tile-0's PV stays bf16: its
            # low-query rows average over too few keys to absorb fp8 noise)
            V_sb = [apool.tile([128, H_PER_CORE * VW], dt.bfloat16,
                               name=f"V{k}", tag=f"V{k}") for k in range(4)]
            # fp8 V in chunk-PAIR layout [128, 2(parity), 4*VW] for the
            # DoubleRow PV matmuls of tiles t>=1 (contract 256 keys per
            # instruction at 0.5 cycles/row: 4x the bf16 pair cost)
            V2_sb = [apool.tile([128, 2, H_PER_CORE * VWP], dt.float8e4,
                                name=f"V2{k}", tag=f"V2{k}")
                     for k in range(KCH // 2)]
            OT_sb = [[apool.tile([128, 512], dt.bfloat16, name=f"OT{j}_{t}",
                                 tag=f"OT{j}_{t}") for t in range(QT_TILES)]
                     for j in range(2)]

            qk_ps = {}

            def emit_qk_proj(w, j, t, c0=0, c1=CCH):
                # t=0: bf16, supports partial c-ranges with a persistent
                # psum group.  t>=1: fp8 DoubleRow -- 4 matmuls each
                # contracting a 256-row c-chunk-pair at 0.5 cycles/row.
                b_sb = (bqs_sb, bks_sb)[w]
                key = (w, j, t)
                if key not in qk_ps:
                    qk_ps[key] = mmpool.tile([128, 512], dt.float32,
                                             name="ps", tag="ps")
                ps = qk_ps[key]
                woff = w * DH + j * 128
                woff8 = 128 * w
                if t == 0:
                    for c in range(c0, c1):
                        nc.tensor.matmul(
                            ps[:],
                            Wqk_sb[:, c, woff:woff + 128],
                            xT_t[t][:, c, :],
                            start=(c == 0), stop=(c == CCH - 1),
                            skip_group_check=True)
                    if c1 < CCH:
                        return
                else:
                    # c0/c1 are cp-pair indices here (0..4): partial ranges
                    # let the tile-1 prefix interleave Q and K chunk-pairs
                    # so both track the split x1 DMA arrivals
                    cp0, cp1 = (c0, min(c1, CCH // 2))
                    for cp in range(cp0, cp1):
                        nc.tensor.matmul(
                            ps[:],
                            Wqk8_sb[j][:, cp, :].rearrange(
                                "p (i d) -> p i d", i=2)[:, :,
                                                         woff8:woff8 + 128],
                            xT_t[t][:, cp, :, :],
                            start=(cp == 0), stop=(cp == CCH // 2 - 1),
                            perf_mode=mybir.MatmulPerfMode.DoubleRow,
                            skip_group_check=True)
                    if cp1 < CCH // 2:
                        return
                if w == 0:
                    d = QT_sb[j][t][:] if t == 0 else QT8_sb[j][t][:, 0, :]
                else:
                    d = KT_sb[j][t][:] if t == 0 else KT8_sb[j][t][:, 0, :]
                nc.vector.tensor_scalar_add(d, ps[:], b_sb[:, j:j + 1])
                del qk_ps[key]

            def emit_k03_f8(j):
                # fp8-projected K chunks 0-3, consumed only by tiles >= 1
                # (their rows average over >=512 keys, so the fp8 projection
                # noise is safe; tile 0 keeps its bf16 K)
                ps = mmpool.tile([128, 512], dt.float32, name="ps", tag="ps")
                for cp in range(CCH // 2):
                    nc.tensor.matmul(
                        ps[:],
                        Wqk8_sb[j][:, cp, :].rearrange(
                            "p (i d) -> p i d", i=2)[:, :, 128:256],
                        x08_sb[:, cp, :, :],
                        start=(cp == 0), stop=(cp == CCH // 2 - 1),
                        perf_mode=mybir.MatmulPerfMode.DoubleRow,
                        skip_group_check=True)
                nc.vector.tensor_scalar_add(KT8_sb[j][0][:, 0, :], ps[:],
                                            bks_sb[:, j:j + 1])

            def emit_v_proj(k):
                ps = mmpool.tile([128, DH], dt.float32, name="ps", tag="ps")
                if k < 4:
                    for c in range(CCH):
                        nc.tensor.matmul(
                            ps[:],
                            xT_t[0][:, c, (k % 4) * 128:(k % 4 + 1) * 128],
                            Wv_sb[:, c, :],
                            start=(c == 0), stop=(c == CCH - 1))
                else:
                    for cp in range(CCH // 2):
                        nc.tensor.matmul(
                            ps[:],
                            xT_t[k // 4][:, cp, :,
                                         (k % 4) * 128:(k % 4 + 1) * 128],
                            Wv8_sb[:, cp, :].rearrange(
                                "p (i e) -> p i e", i=2),
                            start=(cp == 0), stop=(cp == CCH // 2 - 1),
                            perf_mode=mybir.MatmulPerfMode.DoubleRow,
                            skip_group_check=True)
                dsts = [(V2_sb[k // 2][:, k % 2, :], VWP)]
                if k < 4:
                    dsts.append((V_sb[k][:], VW))
                for dst, vw in dsts:
                    v_dst = dst.rearrange("p (h e) -> p h e", e=vw)[:, :, 0:D_K]
                    nc.vector.tensor_tensor(
                        out=v_dst,
                        in0=ps[:].rearrange("p (h e) -> p h e", e=D_K),
                        in1=bvb_sb.rearrange("p (h e) -> p h e", e=D_K),
                        op=mybir.AluOpType.add)
                    od = dst.rearrange("p (h e) -> p h e", e=vw)[:, :, D_K]
                    nc.vector.tensor_scalar(
                        out=od, in0=bqkv_sb[:, 4:4 + H_PER_CORE], scalar1=0.0,
                        scalar2=1.0, op0=mybir.AluOpType.mult,
                        op1=mybir.AluOpType.add)

            # work queues drained into the exp-paced attention stages under
            # a per-stage PE-cost budget: projection groups for future tiles
            # first (they gate those tiles' attention), then output-projection
            # halves.  Tags order the force points: tile t start forces <= t,
            # the first PV flush of group (t, j0) forces <= t+0.5 (V tiles),
            # group (t, j1) start forces <= t+0.75 (its Q/K).
            pending_proj = []   # [tag, pe_cost_ns, fn, key]
            pending_oproj = []

            def force_item(key):
                # run (and remove) every queued item producing `key`, in
                # queue order (split projection groups share a key)
                for it in [x for x in pending_proj if x[3] == key]:
                    pending_proj.remove(it)
                    it[2]()

            def drain_budget(budget, sidx=99, allow_oproj=True,
                             on_act=False, reserve_oproj=0):
                spent = 0
                if budget <= 0:
                    return 0
                while True:
                    # first queued item whose DMA inputs have landed by
                    # this stage (draining earlier would head-of-line
                    # stall the in-order PE on the DMA semaphore)
                    it = next((x for x in pending_proj if x[4] <= sidx),
                              None)
                    if it is None or (spent and spent + it[1] > budget):
                        break
                    pending_proj.remove(it)
                    it[2]()
                    spent += it[1]
                while allow_oproj and len(pending_oproj) > reserve_oproj and spent + OPROJ_COST <= budget:
                    emit_oproj_half(*pending_oproj.pop(0), on_act=on_act)
                    spent += OPROJ_COST
                return spent

            def drain_all(on_act=False):
                while pending_proj:
                    pending_proj.pop(0)[2]()
                if TAIL_PAIR:
                    # merge TWO consecutive output rows into ONE strided
                    # dma (HWDGE costs 625ns per dma_start and the tail's
                    # issue chain ends the kernel); copies split ACT/DVE;
                    # rows alternate between the mm psum bufs and the pv
                    # banks (free after the last normalize)
                    row = 0
                    # reserved leftovers from earlier tiles can sit at the
                    # front half-aligned and would break every merge below:
                    # emit them as singles first
                    while pending_oproj and (
                            pending_oproj[0][2] == 1 or
                            len(pending_oproj) < 2 or
                            pending_oproj[1][1] != pending_oproj[0][1]):
                        emit_oproj_half(*pending_oproj.pop(0),
                                        on_act=(row % 2 == 0))
                        row += 1
                    while len(pending_oproj) >= 4 and \
                            pending_oproj[0][0] == pending_oproj[3][0] and \
                            pending_oproj[0][1] + 1 == pending_oproj[2][1]:
                        (t_, tt, _) = pending_oproj.pop(0)
                        for _ in range(3):
                            pending_oproj.pop(0)
                        o_sb = opool.tile([128, 2, 2, 512], dt.bfloat16,
                                          name="osb4", tag="osb4", bufs=3)
                        for r in range(2):
                            for m in range(2):
                                row += 1
                                if row % 2:
                                    ps = mmpool.tile([128, 512], dt.float32,
                                                     name="ps", tag="ps")
                                else:
                                    ps = pvpool.tile([128, 512], dt.float32,
                                                     name=f"pv{m}",
                                                     tag=f"pv{m}",
                                                     bufs=PV_BUFS)
                                for jj in range(2):
                                    nc.tensor.matmul(
                                        ps[:],
                                        OT_sb[jj][t_][:, ((tt + r) % 4) * 128:
                                                      ((tt + r) % 4 + 1) * 128],
                                        WoT_sb[jj][:, m * 512:(m + 1) * 512],
                                        start=(jj == 0), stop=(jj == 1))
                                if (r + m) % 2 == 0:
                                    nc.scalar.copy(o_sb[:, r, m, :], ps[:])
                                else:
                                    nc.vector.tensor_copy(o_sb[:, r, m, :],
                                                          ps[:])
                        nc.sync.dma_start(
                            out[tt * 128:(tt + 2) * 128, :].rearrange(
                                "(r p) m -> p r m", p=128),
                            o_sb[:])
                    while len(pending_oproj) >= 2 and \
                            pending_oproj[0][1] == pending_oproj[1][1]:
                        (t_, tt, _) = pending_oproj.pop(0)
                        pending_oproj.pop(0)
                        row += 1
                        o_sb = opool.tile([128, 2, 512], dt.bfloat16,
                                          name="osb2", tag="osb2", bufs=4)
                        for m in range(2):
                            if row % 2:
                                ps = mmpool.tile([128, 512], dt.float32,
                                                 name="ps", tag="ps")
                            else:
                                ps = pvpool.tile([128, 512], dt.float32,
                                                 name=f"pv{m}",
                                                 tag=f"pv{m}", bufs=PV_BUFS)
                            for jj in range(2):
                                nc.tensor.matmul(
                                    ps[:],
                                    OT_sb[jj][t_][:, (tt % 4) * 128:
                                                  (tt % 4 + 1) * 128],
                                    WoT_sb[jj][:, m * 512:(m + 1) * 512],
                                    start=(jj == 0), stop=(jj == 1))
                            if m == 0:
                                nc.scalar.copy(o_sb[:, m, :], ps[:])
                            else:
                                nc.vector.tensor_copy(o_sb[:, m, :], ps[:])
                        nc.sync.dma_start(out[tt * 128:(tt + 1) * 128, :],
                                          o_sb[:])
                i = 0
                while pending_oproj:
                    emit_oproj_half(*pending_oproj.pop(0),
                                    on_act=(i % 2 == 0),
                                    use_sc=USC(i))
                    i += 1

            def emit_oproj_half(t, tt, m, on_act=False, use_sc=False):
                # one m-half (512 of 1024 output dims) of a 128-row slice of
                # the output projection: 2 matmuls + psum->sbuf copy + DMA.
                # Half-granularity gives the t3 drain pacing enough
                # resolution to keep every exp-paced stage PE-bound.
                o_sb = opool.tile([128, 512], dt.bfloat16, name="osb",
                                  tag="osb", bufs=OSB_BUFS)
                if use_sc:
                    # endgame: the sc psum pool is idle after the final exp;
                    # borrowing its banks doubles the psum tiles in flight so
                    # the tail matmuls stop waiting on copy-recycled mm bufs
                    ps = scpool.tile([128, 2, 512], dt.float32, name="sc",
                                     tag="sc")[:, 0, :]
                else:
                    ps = mmpool.tile([128, 512], dt.float32, name="ps",
                                     tag="ps")
                for j in range(2):
                    nc.tensor.matmul(
                        ps[:],
                        OT_sb[j][t][:, (tt % 4) * 128:(tt % 4 + 1) * 128],
                        WoT_sb[j][:, m * 512:(m + 1) * 512],
                        start=(j == 0), stop=(j == 1))
                # tail halves copy on ACT (idle after the last exp; DVE is
                # busy with the final normalize chain); halves drained
                # during attention stages copy on DVE (ACT is exp-bound)
                if on_act == "split":
                    # very last half: quarter-split the copy across ACT+DVE
                    # and DMA each quarter out as it lands, shortening the
                    # end-of-kernel copy->DMA->sem chain
                    nc.scalar.copy(o_sb[:, 0:256], ps[:, 0:256])
                    nc.sync.dma_start(
                        out[tt * 128:(tt + 1) * 128,
                            m * 512:m * 512 + 256], o_sb[:, 0:256])
                    nc.vector.tensor_copy(o_sb[:, 256:512], ps[:, 256:512])
                    nc.sync.dma_start(
                        out[tt * 128:(tt + 1) * 128,
                            m * 512 + 256:(m + 1) * 512], o_sb[:, 256:512])
                    return
                if on_act:
                    nc.scalar.copy(o_sb[:], ps[:])
                else:
                    nc.vector.tensor_copy(o_sb[:], ps[:])
                nc.sync.dma_start(
                    out[tt * 128:(tt + 1) * 128, m * 512:(m + 1) * 512],
                    o_sb[:])

            from functools import partial

            def emit_filler():
                for _ in range(N_FILLER):
                    fps = scpool.tile([128, 2, 512], dt.float32, name="sc",
                                      tag="sc")
                    nc.tensor.matmul(fps[:, 0, :], warm_sb[:, 0:128],
                                     warm_sb[:], start=True, stop=True,
                                     skip_group_check=True)

            # global projection work queue, ordered by first use under the
            # [1, 2, 0, 3] tile processing order.  Keys let the attention
            # loop force exactly the group a pair is about to consume.
            TO = [int(c) for c in os.environ.get("K_TO", "1203")]

            def q(tag, cost, fn, key, ready=0):
                pending_proj.append([tag, cost, fn, key, ready])

            # tile-1 j0 prefix, emitted directly in cp-interleaved order:
            # Q/K chunk-pairs 0-1 run on the first x1 half-DMA while
            # chunk-pairs 2-3 wait for the second
            emit_qk_proj(0, 0, 1, 0, 2)
            emit_qk_proj(1, 0, 1, 0, 2)
            emit_qk_proj(0, 0, 1, 2, 4)
            emit_qk_proj(1, 0, 1, 2, 4)
            q(1, 427, partial(emit_k03_f8, 0), ("K8", 0, 0), 1)
            q(1, 427, partial(emit_k03_f8, 1), ("K8", 1, 0), 1)
            q(1, 427, partial(emit_qk_proj, 0, 1, 1), ("Q", 1, 1), 1)
            q(1, 427, partial(emit_qk_proj, 1, 1, 1), ("K", 1, 1), 1)
            for k in (4, 5, 6, 7):
                q(1, 250, partial(emit_v_proj, k), ("V", k), 2)
            for k in (0, 1, 2, 3):
                q(1, 900, partial(emit_v_proj, k), ("V", k), 4)
            q(2, 427, partial(emit_qk_proj, 0, 0, 2), ("Q", 0, 2), 2)
            q(2, 427, partial(emit_qk_proj, 1, 0, 2), ("K", 0, 2), 2)
            for k in (8, 9, 10, 11):
                q(2, 250, partial(emit_v_proj, k), ("V", k), 2)
            q(2, 427, partial(emit_qk_proj, 0, 1, 2), ("Q", 1, 2), 2)
            q(2, 427, partial(emit_qk_proj, 1, 1, 2), ("K", 1, 2), 2)
            q(3, 853, partial(emit_qk_proj, 1, 0, 0, 0, 4), ("K", 0, 0), 5)
            q(3, 900, partial(emit_qk_proj, 1, 0, 0, 4, 8), ("K", 0, 0), 5)
            q(3, 853, partial(emit_qk_proj, 0, 0, 0, 0, 4), ("Q", 0, 0), 6)
            q(3, 900, partial(emit_qk_proj, 0, 0, 0, 4, 8), ("Q", 0, 0), 6)
            q(3, 853, partial(emit_qk_proj, 1, 1, 0, 0, 4), ("K", 1, 0), 5)
            q(3, 900, partial(emit_qk_proj, 1, 1, 0, 4, 8), ("K", 1, 0), 5)
            q(3, 853, partial(emit_qk_proj, 0, 1, 0, 0, 4), ("Q", 1, 0), 6)
            q(3, 900, partial(emit_qk_proj, 0, 1, 0, 4, 8), ("Q", 1, 0), 6)
            q(4, 427, partial(emit_qk_proj, 0, 0, 3), ("Q", 0, 3), 3)
            q(4, 427, partial(emit_qk_proj, 1, 0, 3), ("K", 0, 3), 3)
            for k in (12, 13, 14, 15):
                q(4, 250, partial(emit_v_proj, k), ("V", k), 3)
            q(4, 427, partial(emit_qk_proj, 0, 1, 3), ("Q", 1, 3), 3)
            q(4, 427, partial(emit_qk_proj, 1, 1, 3), ("K", 1, 3), 3)

            for j in range(2):
                nc.sync.dma_start(WoT_sb[j][:],
                                  WoT[j * 128:(j + 1) * 128, :])
            def flush(mms_pp, final):
                for p in range(2):
                    for i, (lhsT, rhs, o, st, pm) in enumerate(mms_pp[p]):
                        nc.tensor.matmul(
                            o, lhsT, rhs, start=st,
                            stop=(final and i == len(mms_pp[p]) - 1),
                            perf_mode=pm, skip_group_check=True)

            pending_norm = []

            def normalize_group(t, j, pvs):
                # OT = PV * (1/denom).  DVE tensor_tensor can read only ONE
                # psum operand, so the broadcast goes through gpsimd to
                # SBUF.  The mults are DEFERRED one stage: an in-order DVE
                # would otherwise head-of-line stall on the Pool broadcast
                # roundtrip, delaying the next group's Q/K bias write.
                bcs = []
                for p in range(2):
                    rc = smpool.tile([1, 512], dt.float32, name="rc",
                                     tag="rc", bufs=SM_BUFS)
                    bc = smpool.tile([64, 512], dt.float32, name="bc",
                                     tag="bc", bufs=SM_BUFS)
                    nc.vector.reciprocal(rc[:], pvs[p][D_K:VW, :])
                    nc.gpsimd.partition_broadcast(bc[:], rc[:])
                    bcs.append(bc)

                def mults():
                    for p in range(2):
                        nc.vector.tensor_tensor(
                            out=OT_sb[j][t][p * 64:(p + 1) * 64, :],
                            in0=pvs[p][0:D_K, :], in1=bcs[p][:],
                            op=mybir.AluOpType.mult)
                pending_norm.append(mults)

            def normalize_last(t, j, pvs):
                # final group: the first 128 columns get their own short
                # recip/bcast/mult so the first tail oproj half (which
                # reads OT cols 0:128) unblocks as early as possible
                rcs = [smpool.tile([1, 512], dt.float32, name="rc",
                                   tag="rc", bufs=SM_BUFS) for _ in range(2)]
                bcs = [smpool.tile([64, 512], dt.float32, name="bc",
                                   tag="bc", bufs=SM_BUFS) for _ in range(2)]
                for hh in range(2):
                    s = slice(hh * 256, (hh + 1) * 256)
                    for p in range(2):
                        nc.vector.reciprocal(rcs[p][:, s], pvs[p][D_K:VW, s])
                        nc.gpsimd.partition_broadcast(bcs[p][:, s],
                                                      rcs[p][:, s])
                    for cc in (2 * hh, 2 * hh + 1):
                        sc_ = slice(cc * 128, (cc + 1) * 128)
                        for p in range(2):
                            nc.vector.tensor_tensor(
                                out=OT_sb[j][t][p * 64:(p + 1) * 64, sc_],
                                in0=pvs[p][0:D_K, sc_], in1=bcs[p][:, sc_],
                                op=mybir.AluOpType.mult)

            # ---- global stage pipeline: one stage per (tile, head-pair,
            # key-chunk-pair).  The PV-flush software pipeline runs ACROSS
            # group and tile boundaries, so a group's flush+normalize tail
            # overlaps the next group's scores/exp instead of serializing
            # at each boundary.  Diagonal pairs first within each group.
            stages = []
            for t in TO:
                for j in range(2):
                    pis = [2 * t, 2 * t + 1] + list(range(2 * t))
                    for pii, pi in enumerate(pis):
                        stages.append((t, j, pi, pii, len(pis)))

            pend = []          # (t, j, pi, mms_pp, pvs, final)
            group_pvs = {}
            LAST = (TO[-1], 1)

            def pop_flush():
                t_, j_, pi_, mms_, pvs_, final_ = pend.pop(0)
                force_item(("V", 2 * pi_))
                force_item(("V", 2 * pi_ + 1))
                flush(mms_, final_)
                if final_:
                    if (t_, j_) == LAST:
                        normalize_last(t_, j_, pvs_)
                    else:
                        normalize_group(t_, j_, pvs_)
                    if j_ == 1:
                        # both head-pairs' OT ready: queue the tile's
                        # output-projection halves
                        for tt in range(4 * t_, 4 * t_ + 4):
                            for m in range(2):
                                pending_oproj.append((t_, tt, m))

            for sidx, (t, j, pi, pii, npair) in enumerate(stages):
                if pii == 0:
                    if (t, j) == (TO[0], 0):
                        # head-critical zero-fills: tile 1's own k-tile
                        # padding plus the chunk-0-3 K it reads
                        for jj in range(2):
                            memset_pad(QT8_sb[jj][1], ("q", jj, 1))
                            memset_pad(KT8_sb[jj][1], ("k", jj, 1))
                            memset_pad(KT8_sb[jj][0], ("k", jj, 0))
                    elif j == 1 and t in (TO[0], TO[1]):
                        # prefetch the NEXT tile's pads a whole group early
                        # so Pool has slack to run them behind the masks
                        tn = TO[TO.index(t) + 1]
                        for jj in range(2):
                            if tn > 0:
                                memset_pad(QT8_sb[jj][tn], ("q", jj, tn))
                            memset_pad(KT8_sb[jj][tn], ("k", jj, tn))
                    elif j == 0 and t == TO[2]:
                        for jj in range(2):
                            memset_pad(QT8_sb[jj][3], ("q", jj, 3))
                            memset_pad(KT8_sb[jj][3], ("k", jj, 3))
                    # this group's own Q/K must exist before its scores
                    force_item(("Q", j, t))
                    force_item(("K", j, t))
                    group_pvs[(t, j)] = [
                        pvpool.tile([128, 512], dt.float32, name=f"pv{p}",
                                    tag=f"pv{p}", bufs=PV_BUFS)
                        for p in range(2)]
                pvs = group_pvs[(t, j)]
                diag = pi >= 2 * t
                first_pair = pii == 0
                # for the second diagonal pair (r0=2) only columns
                # q >= 128*r0 can be unmasked for either half, so the
                # scores matmuls and exp skip the dead columns.
                q0 = 128 * 2 * (pi - 2 * t) if diag else 0
                use8 = t > 0  # fp8 DoubleRow scores+PV for tiles >= 1
                pt_dt = dt.float8e4 if use8 else dt.bfloat16
                pt_pool = pt8pool if use8 else ptpool
                # the K projection tile this pair's scores read
                if use8 and pi // 2 == 0:
                    force_item(("K8", j, 0))
                else:
                    force_item(("K", j, pi // 2))
                pts = []
                for p in range(2):
                    lo = p * 64
                    sc = scpool.tile([128, 2, 512], dt.float32,
                                     name="sc", tag="sc")
                    for half in range(2):
                        c = 2 * pi + half
                        # per-half trim: k-block r = 2(pi-2t)+half is only
                        # unmasked for q >= 128*r
                        q0h = 128 * (2 * (pi - 2 * t) + half) \
                            if diag else 0
                        if use8:
                            nc.tensor.matmul(
                                sc[:, half, q0h:],
                                KT8_sb[j][(2 * pi) // 4][
                                    lo:lo + 64, :,
                                    (c % 4) * 128:(c % 4 + 1) * 128],
                                QT8_sb[j][t][lo:lo + 64, :, q0h:],
                                start=True, stop=True,
                                tile_position=(lo, 0),
                                perf_mode=mybir.MatmulPerfMode.DoubleRow)
                        else:
                            nc.tensor.matmul(
                                sc[:, half, q0h:],
                                KT_sb[j][(2 * pi) // 4][
                                    lo:lo + 64,
                                    (c % 4) * 128:(c % 4 + 1) * 128],
                                QT_sb[j][t][lo:lo + 64, q0h:],
                                start=True, stop=True,
                                tile_position=(lo, 0))
                    pt = pt_pool.tile([128, 2, 512], pt_dt,
                                      name="pt", tag="pt")
                    nc.scalar.activation(pt[:, :, q0:],
                                         sc[:, :, q0:], AF.Exp)
                    pts.append(pt)
                mms_pp = []
                for p in range(2):
                    pt = pts[p]
                    h = 2 * j + p
                    vsl = lambda c: V_sb[c][:, h * VW:(h + 1) * VW]
                    mms = []  # (lhsT, rhs, out, start, perf_mode)
                    if use8:
                        v2 = V2_sb[pi][:, :, h * VWP:h * VWP + VW]
                        if diag:
                            r0 = 2 * (pi - 2 * t)
                            # half0: zero below-diagonal of its 128x128
                            # triangle block
                            tri = pt[:, 0, 128 * r0:128 * (r0 + 1)]
                            nc.gpsimd.affine_select(
                                out=tri, in_=tri,
                                compare_op=mybir.AluOpType.is_ge,
                                fill=0.0, base=0, pattern=[[1, 128]],
                                channel_multiplier=-1)
                            # half1: cols [128r0, 128(r0+1)) are entirely
                            # below-diagonal plus its own triangle block --
                            # one widened select covers both
                            tri2 = pt[:, 1, 128 * r0:128 * (r0 + 2)]
                            nc.gpsimd.affine_select(
                                out=tri2, in_=tri2,
                                compare_op=mybir.AluOpType.is_ge,
                                fill=0.0, base=-128, pattern=[[1, 256]],
                                channel_multiplier=-1)
                            mms.append((v2, pt[:, :, 128 * r0:],
                                        pvs[p][0:VW, 128 * r0:],
                                        first_pair,
                                        mybir.MatmulPerfMode.DoubleRow))
                        else:
                            mms.append((v2, pt[:, :, :],
                                        pvs[p][0:VW, :], first_pair,
                                        mybir.MatmulPerfMode.DoubleRow))
                    elif not diag:
                        for half in range(2):
                            mms.append((vsl(2 * pi + half),
                                        pt[:, half, :], pvs[p][0:VW, :],
                                        first_pair and half == 0, None))
                    else:
                        r0 = 2 * (pi - 2 * t)
                        for half in range(2):
                            r = r0 + half
                            tri = pt[:, half, 128 * r:128 * (r + 1)]
                            nc.gpsimd.affine_select(
                                out=tri, in_=tri,
                                compare_op=mybir.AluOpType.is_ge,
                                fill=0.0, base=0, pattern=[[1, 128]],
                                channel_multiplier=-1)
                        for half in range(2):
                            r = r0 + half
                            mms.append((vsl(2 * pi + half),
                                        pt[:, half, 128 * r:],
                                        pvs[p][0:VW, 128 * r:],
                                        first_pair and half == 0, None))
                    mms_pp.append(mms)
                pend.append((t, j, pi, mms_pp, pvs, pii == npair - 1))
                while pending_norm:
                    pending_norm.pop(0)()
                drain_budget(BUD_T0 if t == 0 else BUD, sidx=sidx,
                             reserve_oproj=RESERVE)
                # the LAST group flushes shallow: with the full depth its
                # pairs would all queue behind the final exp, serializing
                # flush+normalize+output-projection into the tail
                depth = PV_LAST if (t, j) == LAST else PV_DEPTH
                while len(pend) > depth:
                    pop_flush()
            # pipeline tail: flush the last pairs, then the reserved oproj
            # halves fill the final normalize window
            while pend:
                pop_flush()
                while pending_norm:
                    pending_norm.pop(0)()
            drain_all()
            if TAIL_PAIR:
                # tail: one o_sb + one DMA per tt row-block (the 8 tail
                # DMAs otherwise serialize on the 625ns-per-DMA HWDGE)
                for i in range(0, len(pending_oproj), 2):
                    (t_, tt, _), _ = pending_oproj[i], pending_oproj[i + 1]
                    o_sb = opool.tile([128, 2, 512], dt.bfloat16,
                                      name="osb2", tag="osb2", bufs=2)
                    for m in range(2):
                        ps = mmpool.tile([128, 512], dt.float32, name="ps",
                                         tag="ps")
                        for j in range(2):
                            nc.tensor.matmul(
                                ps[:],
                                OT_sb[j][t_][:,
                                             (tt % 4) * 128:(tt % 4 + 1) * 128],
                                WoT_sb[j][:, m * 512:(m + 1) * 512],
                                start=(j == 0), stop=(j == 1))
                        if m == 0:
                            nc.scalar.copy(o_sb[:, m, :], ps[:])
                        else:
                            nc.vector.tensor_copy(o_sb[:, m, :], ps[:])
                    nc.sync.dma_start(out[tt * 128:(tt + 1) * 128, :],
                                      o_sb[:])
            else:
                for i, args in enumerate(pending_oproj):
                    last = i == len(pending_oproj) - 1
                    emit_oproj_half(*args,
                                    on_act="split" if (last and SPLIT_LAST)
                                    else OACT(i),
                                    use_sc=USC(i))
    nc.compile()
    return nc


def _in_maps(x, Wq, bq, Wk, bk, Wv, bv, Wo, bo):
    maps = []
    F8 = ml_dtypes.float8_e4m3fn
    dts = {"xT0": BF16, "xT8": F8, "x08T": F8, "WqkT": BF16,
           "Wqk8AT": F8, "Wqk8BT": F8,
           "WvT": BF16, "Wv8T": F8, "WoT": BF16, "bqkv": np.float32}
    for core in range(N_CORES):
        b = core // 4
        h0 = (core % 4) * H_PER_CORE
        hs = slice(h0 * D_K, (h0 + H_PER_CORE) * D_K)
        xTb = np.ascontiguousarray(x[b].T)
        wqk = np.concatenate([
            np.ascontiguousarray(Wq[hs, :].T) * 0.125,
            np.ascontiguousarray(Wk[hs, :].T)], axis=1)
        wv = np.ascontiguousarray(Wv[hs, :].T)
        m = {
            "xT0": xTb[:, 0:512],
            "xT8": xTb[:, 512:],
            "x08T": xTb[:, 0:512],
            "WqkT": wqk,
            "Wqk8AT": np.concatenate([wqk[:, 0:128], wqk[:, 256:384]],
                                     axis=1).reshape(
                CCH // 2, 2, 128, 256).transpose(0, 2, 1, 3).reshape(
                D_MODEL // 2, 512),
            "Wqk8BT": np.concatenate([wqk[:, 128:256], wqk[:, 384:512]],
                                     axis=1).reshape(
                CCH // 2, 2, 128, 256).transpose(0, 2, 1, 3).reshape(
                D_MODEL // 2, 512),
            "WvT": wv,
            "Wv8T": wv.reshape(CCH // 2, 2, 128, DH).transpose(
                0, 2, 1, 3).reshape(D_MODEL // 2, 2 * DH),
            "WoT": np.ascontiguousarray(Wo[:, hs].T),
            "bqkv": np.concatenate([
                np.ascontiguousarray((bq[hs] * 0.125).reshape(2, 128).T),
                np.ascontiguousarray(bk[hs].reshape(2, 128).T),
                np.broadcast_to(bv[hs], (128, DH)),
            ], axis=1),
        }
        maps.append({k: np.ascontiguousarray(v, dtype=dts[k])
                     for k, v in m.items()})
    return maps


def kernel(x, Wq, bq, Wk, bk, Wv, bv, Wo, bo, _trace=False):
    if "nc" not in _CACHE:
        _CACHE["nc"] = _build()
    nc = _CACHE["nc"]
    in_maps = _in_maps(np.asarray(x, dtype=np.float32),
                       np.asarray(Wq, dtype=np.float32),
                       np.asarray(bq, dtype=np.float32),
                       np.asarray(Wk, dtype=np.float32),
                       np.asarray(bk, dtype=np.float32),
                       np.asarray(Wv, dtype=np.float32),
                       np.asarray(bv, dtype=np.float32),
                       np.asarray(Wo, dtype=np.float32),
                       np.asarray(bo, dtype=np.float32))
    res = run_bass_kernel_spmd(nc, in_maps, core_ids=list(range(N_CORES)),
                               trace=_trace)
    bo = np.asarray(bo, dtype=np.float32)
    out = np.zeros((B, S, D_MODEL), dtype=np.float32)
    for b in range(B):
        acc = res.results[b * 4]["out"].astype(np.float64)
        for core in range(b * 4 + 1, b * 4 + 4):
            acc = acc + res.results[core]["out"]
        out[b] = (acc + bo).astype(np.float32)
    if _trace:
        return out, res
    return out

